# revision 1
# baseline (speedup 1.0000x reference)
"""Trainium2 Bass kernel for the CAA (channel-affinity attention) module.

Reference computation per sample b (C=1024 channels, N=256 positions):
    x_hat = x^T                              (N, C)
    q = relu(BN1(Wq @ x_hat))                (64, C)
    k = relu(BN2(Wk @ x_hat))                (64, C)
    sim[c, d] = sum_o k[o, c] * q[o, d]      (C, C)
    aff = softmax(rowmax(sim) - sim, axis d) == softmax(-sim, axis d)
    v = relu(BN3(Wv @ x))                    (C, N)
    out = alpha * (aff @ v) + x              (C, N)

Device-side strategy (pure data parallel, 4 samples per core x 8 cores):
  * BN folded into weights/bias on the host.
  * sim is computed TRANSPOSED (d on partitions, c on free) so the exp(-sim)
    tiles feed the aff @ v contraction directly as matmul lhsT.
  * sim uses 2-way PE row tiling: the K=64 contraction only fills half the
    array, so two d-chunks run concurrently in rows 0-63 / 64-127 (needs k
    copied to partitions 0-63 and q to 64-127 -- one SBUF-SBUF DMA each).
  * the qk projection, v = relu(Wv'x + t3), and U = E @ v_ext all run in
    fp8 (e4m3) with DoubleRow perf mode, which packs two contraction rows
    per PE cell (2x throughput). v packs two samples side by side in the
    moving operand so each matmul streams 512 virtual columns per weight
    load; U pairs adjacent d-chunks via [p, 2, free] access patterns.
  * exp tiles are 16*exp(-sim) in fp8 (ACT LUT, bias=ln16): the 16x keeps
    e4m3 from underflowing whole softmax rows (row-min sim <= ~10), and
    the same factor enters Z through the 16/alpha ones-columns, so it
    cancels in U * (1/Zcol). Wv carries a further 16x for the same reason.
  * epilogues: v bias+relu and the fused U*(1/Z)+x reside on the DVE
    (reciprocal_approx_fast for 1/Z); exp owns the ACT; the PE clock is
    kept warm through the initial DMA wait by dummy matmuls.
  * the residual add reads the bf16 copy of x; output is stored bf16.
"""

import os
import sys

import numpy as np

_REPO = "/opt/trn_rl_repo"
if _REPO not in sys.path:
    sys.path.insert(0, _REPO)

import ml_dtypes  # noqa: E402

import concourse.bacc as bacc  # noqa: E402
import concourse.tile as tile  # noqa: E402
from concourse import mybir  # noqa: E402
from concourse.bass_utils import run_bass_kernel_spmd  # noqa: E402

F32 = mybir.dt.float32
BF16 = mybir.dt.bfloat16
FP8 = mybir.dt.float8e4
AF = mybir.ActivationFunctionType
ALU = mybir.AluOpType
PM = mybir.MatmulPerfMode
BFNP = ml_dtypes.bfloat16
F8NP = ml_dtypes.float8_e4m3

B, C, N = 32, 1024, 256
DQ = 64
NCORES = 8
BS = B // NCORES  # samples per core
CCH = C // 128    # chunks of the channel dim
KCH = N // 128    # chunks of the position dim (qk contraction)
EPS = 1e-5
WV_SCALE = 16.0   # keeps fp8 Wv weights in e4m3 normal range
E_SCALE_LN = 2.772588722239781  # ln(16): exp tiles carry a 16x factor
NP = 264          # padded v free size (fp8 DoubleRow AP needs %16 strides)
LAST_RESULTS = None  # BassKernelResults of the most recent run
_NC_CACHE = {}


def _build(bs: int = BS):
    nc = bacc.Bacc("TRN2", target_bir_lowering=False, debug=False)

    xb_d = nc.dram_tensor("xb_in", (bs, 128, CCH, N), BF16, kind="ExternalInput")
    xt_d = nc.dram_tensor("xt_in", (bs, 128, KCH, C), FP8, kind="ExternalInput")
    # fp8 x with two samples interleaved at the n level: [g, p, kc, j, n]
    xb8_d = nc.dram_tensor("xb8_in", (bs // 2, 128, CCH, 2, N), FP8,
                           kind="ExternalInput")
    wqkt_d = nc.dram_tensor("wqkt", (128, KCH, 128), FP8, kind="ExternalInput")
    tqk_d = nc.dram_tensor("tqk", (128, 1), F32, kind="ExternalInput")
    wvt_d = nc.dram_tensor("wvt", (128, CCH, C), FP8, kind="ExternalInput")
    t3_d = nc.dram_tensor("t3", (128, CCH), F32, kind="ExternalInput")
    vcol_d = nc.dram_tensor("vcol", (128, CCH, 2, 8), BF16, kind="ExternalInput")
    out_d = nc.dram_tensor("y_out", (bs, 128, CCH, N), BF16, kind="ExternalOutput")

    with tile.TileContext(nc) as tc:
        with (
            tc.tile_pool(name="consts", bufs=1) as consts,
            tc.tile_pool(name="xbp", bufs=4) as xbp,
            tc.tile_pool(name="xtp", bufs=2) as xtp,
            tc.tile_pool(name="x8p", bufs=2) as x8p,
            tc.tile_pool(name="qkp", bufs=2) as qkp,
            tc.tile_pool(name="qsp", bufs=2) as qsp,
            tc.tile_pool(name="etp", bufs=10) as etp,
            tc.tile_pool(name="etmp", bufs=2) as etmpp,
            tc.tile_pool(name="vp", bufs=2) as vp,
            tc.tile_pool(name="outp", bufs=2) as outp,
            tc.tile_pool(name="smallp", bufs=8) as smallp,
            tc.tile_pool(name="psbig", bufs=2, space="PSUM") as psbig,
            tc.tile_pool(name="psv", bufs=2, space="PSUM") as psvp,
            tc.tile_pool(name="psbank", bufs=2, space="PSUM") as psbank,
        ):
            # small weights via SWDGE (gpsimd) so they don't block the sync
            # queue; wvt8 is chunked so its first bytes land early
            wvt8 = consts.tile([128, CCH, C], FP8, tag="wvt8")
            for co in range(0, C, 512):
                nc.gpsimd.dma_start(out=wvt8[:, :, co:co + 512],
                                    in_=wvt_d[:, :, co:co + 512])
            wqkt = consts.tile([128, KCH, 128], FP8, tag="wqkt")
            nc.gpsimd.dma_start(out=wqkt, in_=wqkt_d[:])
            tqk = consts.tile([128, 1], F32, tag="tqk")
            nc.gpsimd.dma_start(out=tqk, in_=tqk_d[:])
            t3 = consts.tile([128, CCH], F32, tag="t3")
            nc.gpsimd.dma_start(out=t3, in_=t3_d[:])
            vcol4 = consts.tile([128, CCH, 2, 8], BF16, tag="vcol")
            nc.gpsimd.dma_start(out=vcol4, in_=vcol_d[:])
            zero = consts.tile([128, 1], F32, tag="zero")
            nc.vector.memset(zero, 0.0)
            ln16 = consts.tile([128, 1], F32, tag="ln16")
            nc.vector.memset(ln16, E_SCALE_LN)
            # touch the activation table early so the lazy ACT_TABLE_LOAD
            # doesn't delay the first (critical-path) exp
            warm = consts.tile([128, 1], F32, tag="warm")
            nc.scalar.activation(out=warm, in_=zero, func=AF.Exp,
                                 bias=zero[:, 0:1], scale=1.0)
            # dummy-matmul fodder: keeps the PE busy through the initial DMA
            # wait so the HAM clock gate reaches 8/8 before the real work
            zwarm = consts.tile([128, 256], BF16, tag="zwarm")
            nc.vector.memset(zwarm, 0.0)

            def warm_burst(n):
                warm_ps = psbank.tile([128, NP], F32, tag="psbank")
                for _ in range(n):
                    nc.tensor.matmul(
                        warm_ps[:, 0:256],
                        zwarm[:, 0:128],
                        zwarm,
                        start=True,
                        stop=True,
                    )

            xb_sb = [None] * bs
            xt_sb = [None] * bs
            x8_sb = [None] * (bs // 2)
            qk_sb = [None] * bs
            qsw = [None] * bs
            v2_sb = [None] * (bs // 2)
            et = [None] * bs

            def load_xt(b):
                xt_sb[b] = xtp.tile([128, KCH, C], FP8, tag="xt",
                                    name=f"xt_sb{b}")
                nc.sync.dma_start(out=xt_sb[b], in_=xt_d[b])

            def load_xb(b):
                xb_sb[b] = xbp.tile([128, CCH, N], BF16, tag="xb",
                                    name=f"xb_sb{b}")
                nc.sync.dma_start(out=xb_sb[b], in_=xb_d[b])

            def load_x8(g):
                x8_sb[g] = x8p.tile([128, CCH, 2, N], FP8, tag="x8",
                                    name=f"x8_{g}")
                nc.sync.dma_start(out=x8_sb[g][:, 0:CCH // 2], in_=xb8_d[g, :, 0:CCH // 2])
                nc.sync.dma_start(out=x8_sb[g][:, CCH // 2:], in_=xb8_d[g, :, CCH // 2:])

            def qk_phase(b):
                # q/k projection: psum rows 0:64 = q, 64:128 = k
                # qk psum comes from the psv pool so the sim-pair pool
                # (psbig) never gates the next sample's projection
                qk_sb[b] = qkp.tile([128, C], BF16, tag="qk", name=f"qk_sb{b}")
                for cb in range(C // 512):
                    qk_ps = psvp.tile([128, 512], F32, tag="psv")
                    nc.tensor.matmul(
                        qk_ps,
                        wqkt,
                        xt_sb[b][:, :, cb * 512:(cb + 1) * 512],
                        start=True,
                        stop=True,
                        perf_mode=PM.DoubleRow,
                    )
                    # bias+relu on the DVE: keeps the latency-critical relu
                    # out of the ACT FIFO (which is busy draining exp tiles)
                    nc.vector.tensor_scalar(
                        out=qk_sb[b][:, cb * 512:(cb + 1) * 512], in0=qk_ps,
                        scalar1=tqk[:, 0:1], scalar2=0.0,
                        op0=ALU.add, op1=ALU.max,
                    )
                # row-tiled sim needs k at partitions 0:63 and a second copy
                # of q at 64:127; the scalar ring is idle here so the copies
                # issue promptly after the relu
                qsw[b] = qsp.tile([128, C], BF16, tag="qsw", name=f"qsw{b}")
                nc.scalar.dma_start(out=qsw[b][0:64, :], in_=qk_sb[b][64:128, :])
                nc.scalar.dma_start(out=qsw[b][64:128, :], in_=qk_sb[b][0:64, :])

            def sim_phase(b):
                # transposed sim + exp: et[d][dd, c] = exp(-sim[c, d]).
                # two d-chunks run concurrently via PE row tiling (K=64 each).
                et[b] = []
                for dp in range(CCH // 2):
                    d0, d1 = 2 * dp, 2 * dp + 1
                    psA = psbig.tile([128, C], F32, tag="psbig")
                    psB = psbig.tile([128, C], F32, tag="psbig")
                    for cb in range(C // 512):
                        sl = slice(cb * 512, (cb + 1) * 512)
                        nc.tensor.matmul(
                            psA[:, sl],
                            qk_sb[b][0:64, d0 * 128:(d0 + 1) * 128],
                            qsw[b][0:64, sl],
                            start=True, stop=True,
                        )
                        nc.tensor.matmul(
                            psB[:, sl],
                            qsw[b][64:128, d1 * 128:(d1 + 1) * 128],
                            qk_sb[b][64:128, sl],
                            start=True, stop=True,
                        )
                    e2 = etp.tile([128, 2, C], FP8, tag="et")
                    for j, ps in ((0, psA), (1, psB)):
                        # 16*exp(-sim) on the ACT LUT (bias = ln 16)
                        nc.scalar.activation(
                            out=e2[:, j, :], in_=ps, func=AF.Exp,
                            bias=ln16[:, 0:1], scale=-1.0,
                        )
                    et[b].append(e2)

            def v_group(g):
                # v for samples 2g/2g+1 in fp8 DoubleRow: rhs [p, kcpair, j, n]
                # streams 512 virtual columns per weight load. v carries a
                # WV_SCALE factor; the 32/alpha ones-columns put the same
                # factor into Z so U*(1/Z) comes out as alpha*(aff @ v).
                v2_sb[g] = vp.tile([128, CCH, 2, NP], FP8, tag="v",
                                   name=f"v2_{g}")
                nc.gpsimd.dma_start(out=v2_sb[g][:, :, :, N:NP], in_=vcol4[:])
                for m in range(CCH):
                    psv = psvp.tile([128, 2 * N], F32, tag="psv")
                    for kcp in range(CCH // 2):
                        nc.tensor.matmul(
                            psv,
                            wvt8[:, 2 * kcp:2 * kcp + 2, m * 128:(m + 1) * 128],
                            x8_sb[g][:, 2 * kcp:2 * kcp + 2, :, :],
                            start=(kcp == 0),
                            stop=(kcp == CCH // 2 - 1),
                            perf_mode=PM.DoubleRow,
                        )
                    nc.vector.tensor_scalar(
                        out=v2_sb[g][:, m, :, 0:N],
                        in0=psv,
                        scalar1=t3[:, m:m + 1],
                        scalar2=0.0,
                        op0=ALU.add,
                        op1=ALU.max,
                    )

            def u_phase(b):
                # U = E @ v_ext (col N accumulates (32/alpha)*Z), then
                # out = U * (1/Zcol) + x fused on the DVE (x residual in bf16)
                o_sb = outp.tile([128, CCH, N], BF16, tag="o")
                for m in range(CCH):
                    u_ps = psbank.tile([128, NP], F32, tag="psbank")
                    for dp in range(CCH // 2):
                        nc.tensor.matmul(
                            u_ps,
                            et[b][dp][:, :, m * 128:(m + 1) * 128],
                            v2_sb[b // 2][:, 2 * dp:2 * dp + 2, b % 2, :],
                            start=(dp == 0),
                            stop=(dp == CCH // 2 - 1),
                            perf_mode=PM.DoubleRow,
                        )
                    rz = smallp.tile([128, 1], F32, tag="rz")
                    nc.vector.reciprocal_approx_fast(out=rz, in_=u_ps[:, N:N + 1])
                    nc.vector.scalar_tensor_tensor(
                        out=o_sb[:, m, :],
                        in0=u_ps[:, 0:N],
                        scalar=rz[:, 0:1],
                        in1=xb_sb[b][:, m, :],
                        op0=ALU.mult,
                        op1=ALU.add,
                    )
                    if m % 2 == 1:
                        # stream the result out in 2-chunk pieces so the
                        # store overlaps the remaining compute
                        nc.sync.dma_start(
                            out=out_d[b, :, m - 1:m + 1, :],
                            in_=o_sb[:, m - 1:m + 1, :],
                        )

            # loads ordered so sample 0/1 tensors land first; dummy matmuls
            # bridge the initial DMA wait and keep the HAM clock warm
            load_xt(0)
            load_x8(0)
            load_xt(1)
            load_xb(0)
            load_xb(1)
            warm_burst(18)
            qk_phase(0)
            warm_burst(6)
            v_group(0)
            sim_phase(0)
            qk_phase(1)
            sim_phase(1)
            u_phase(0)
            load_xt(2)
            load_xb(2)
            load_x8(1)
            load_xt(3)
            load_xb(3)
            qk_phase(2)
            sim_phase(2)
            u_phase(1)
            qk_phase(3)
            sim_phase(3)
            # v for samples 2/3 is deliberately late: its matmuls fill the
            # PE while sample 3's exp tiles drain, so u(2)/u(3) run densely
            v_group(1)
            u_phase(2)
            u_phase(3)

    nc.compile()
    return nc


def _prep_host(x, Wq, Wk, Wv, bn1_g, bn1_b, bn1_m, bn1_v,
               bn2_g, bn2_b, bn2_m, bn2_v, bn3_g, bn3_b, bn3_m, bn3_v):
    f = np.float32
    s1 = (bn1_g / np.sqrt(bn1_v + EPS)).astype(f)
    t1 = (bn1_b - s1 * bn1_m).astype(f)
    s2 = (bn2_g / np.sqrt(bn2_v + EPS)).astype(f)
    t2 = (bn2_b - s2 * bn2_m).astype(f)
    s3 = (bn3_g / np.sqrt(bn3_v + EPS)).astype(f)
    t3 = ((bn3_b - s3 * bn3_m) * WV_SCALE).astype(f)

    wqk = np.concatenate([Wq * s1[:, None], Wk * s2[:, None]], axis=0).astype(f)
    # lhsT layout [p(=n local), kc, o], fp8
    wqkt = np.ascontiguousarray(
        wqk.T.reshape(KCH, 128, 128).transpose(1, 0, 2)).astype(F8NP)
    tqk = np.concatenate([t1, t2]).reshape(128, 1).astype(f)

    wv2 = (Wv * (s3 * WV_SCALE)[:, None]).astype(f)
    # lhsT layout [p(=ci local), kc, co], fp8
    wvt8 = np.ascontiguousarray(
        wv2.T.reshape(CCH, 128, C).transpose(1, 0, 2)).astype(F8NP)
    t3r = np.ascontiguousarray(t3.reshape(CCH, 128).T)

    x = np.asarray(x, dtype=f)
    # [b, p(=c local), kc, n]
    xq = x.reshape(B, CCH, 128, N).transpose(0, 2, 1, 3)
    xb = np.ascontiguousarray(xq).astype(BFNP)
    # [g, p, kc, j, n] fp8 pairs for the DoubleRow v matmul
    xb8 = np.ascontiguousarray(
        xq.reshape(B // 2, 2, 128, CCH, N).transpose(0, 2, 3, 1, 4)
    ).astype(F8NP)
    # [b, p(=n local), kc, c], fp8 for the DoubleRow qk matmul
    xt = np.ascontiguousarray(
        x.transpose(0, 2, 1).reshape(B, KCH, 128, C).transpose(0, 2, 1, 3)
    ).astype(F8NP)
    return xb, xb8, xt, wqkt, tqk, wvt8, t3r


def kernel(x, Wq, Wk, Wv,
           bn1_g, bn1_b, bn1_m, bn1_v,
           bn2_g, bn2_b, bn2_m, bn2_v,
           bn3_g, bn3_b, bn3_m, bn3_v,
           alpha):
    global LAST_RESULTS
    args = [np.asarray(a, dtype=np.float32) for a in (
        x, Wq, Wk, Wv, bn1_g, bn1_b, bn1_m, bn1_v,
        bn2_g, bn2_b, bn2_m, bn2_v, bn3_g, bn3_b, bn3_m, bn3_v)]
    alpha_val = float(np.asarray(alpha).reshape(-1)[0])
    if alpha_val == 0.0:
        return np.asarray(x, dtype=np.float32).copy()

    xb, xb8, xt, wqkt, tqk, wvt8, t3r = _prep_host(*args)
    # the Z column accumulates (WV_SCALE/alpha)*Z, so U*(1/Zcol) yields
    # alpha*(aff @ v) with both the fp8 weight scale and alpha folded in
    vcol = np.zeros((128, CCH, 2, 8), dtype=BFNP)
    vcol[:, :, :, 0:2] = np.asarray(WV_SCALE / alpha_val, dtype=BFNP)

    if "nc" not in _NC_CACHE:
        _NC_CACHE["nc"] = _build()
    nc = _NC_CACHE["nc"]

    in_maps = []
    for cid in range(NCORES):
        sl = slice(cid * BS, (cid + 1) * BS)
        slg = slice(cid * (BS // 2), (cid + 1) * (BS // 2))
        in_maps.append({
            "xb_in": np.ascontiguousarray(xb[sl]),
            "xb8_in": np.ascontiguousarray(xb8[slg]),
            "xt_in": np.ascontiguousarray(xt[sl]),
            "wqkt": wqkt,
            "tqk": tqk,
            "wvt": wvt8,
            "t3": t3r,
            "vcol": vcol,
        })

    trace = bool(int(os.environ.get("KERNEL_TRACE", "0")))
    tmpdir = os.environ.get("KERNEL_TRACE_DIR") or None
    res = run_bass_kernel_spmd(
        nc, in_maps, core_ids=list(range(NCORES)), trace=trace, tmpdir=tmpdir
    )
    LAST_RESULTS = res

    y = np.concatenate(
        [np.asarray(res.results[cid]["y_out"], dtype=np.float32)
         for cid in range(NCORES)], axis=0)
    y = y.transpose(0, 2, 1, 3).reshape(B, C, N)
    return np.ascontiguousarray(y)



# revision 4
# speedup vs baseline: 1.1316x; 1.1316x over previous
"""Trainium2 Bass kernel for the CAA (channel-affinity attention) module.

Reference computation per sample b (C=1024 channels, N=256 positions):
    x_hat = x^T                              (N, C)
    q = relu(BN1(Wq @ x_hat))                (64, C)
    k = relu(BN2(Wk @ x_hat))                (64, C)
    sim[c, d] = sum_o k[o, c] * q[o, d]      (C, C)
    aff = softmax(rowmax(sim) - sim, axis d) == softmax(-sim, axis d)
    v = relu(BN3(Wv @ x))                    (C, N)
    out = alpha * (aff @ v) + x              (C, N)

Device-side strategy (pure data parallel, 4 samples per core x 8 cores):
  * BN folded into weights/bias on the host.
  * sim is computed TRANSPOSED (d on partitions, c on free) so the exp(-sim)
    tiles feed the aff @ v contraction directly as matmul lhsT.
  * sim uses 2-way PE row tiling: the K=64 contraction only fills half the
    array, so two d-chunks run concurrently in rows 0-63 / 64-127 (needs k
    copied to partitions 0-63 and q to 64-127 -- one SBUF-SBUF DMA each).
  * the qk projection, v = relu(Wv'x + t3), and U = E @ v_ext all run in
    fp8 (e4m3) with DoubleRow perf mode, which packs two contraction rows
    per PE cell (2x throughput). v packs two samples side by side in the
    moving operand so each matmul streams 512 virtual columns per weight
    load; U pairs adjacent d-chunks via [p, 2, free] access patterns.
  * exp tiles are 16*exp(-sim) in fp8 (ACT LUT, bias=ln16): the 16x keeps
    e4m3 from underflowing whole softmax rows (row-min sim <= ~10), and
    the same factor enters Z through the 16/alpha ones-columns, so it
    cancels in U * (1/Zcol). Wv carries a further 16x for the same reason.
  * epilogues: v bias+relu and the fused U*(1/Z)+x reside on the DVE
    (reciprocal_approx_fast for 1/Z); exp owns the ACT; the PE clock is
    kept warm through the initial DMA wait by dummy matmuls.
  * the residual add reads the bf16 copy of x; output is stored bf16.

Scheduling (v2): all weights load FIRST on the HW sync queue (the old
SWDGE path delivered wqkt at ~17us and stalled the first projection), and
the per-sample phases are software-pipelined at chunk granularity: sim
dp-chunks are interleaved with u/v m-chunks in program order so the PE
engine queue always has >~2.3us of independent work between consecutive
sim chunks (psbig has 2 PSUM bufs; sim chunk t+1 must wait for ACT to
drain chunk t's exps -- FIFO queues stall everything behind it otherwise).
"""

import os
import sys

import numpy as np

_REPO = "/opt/trn_rl_repo"
if _REPO not in sys.path:
    sys.path.insert(0, _REPO)

import ml_dtypes  # noqa: E402

import concourse.bacc as bacc  # noqa: E402
import concourse.tile as tile  # noqa: E402
from concourse import mybir  # noqa: E402
from concourse.bass_utils import run_bass_kernel_spmd  # noqa: E402

F32 = mybir.dt.float32
BF16 = mybir.dt.bfloat16
FP8 = mybir.dt.float8e4
AF = mybir.ActivationFunctionType
ALU = mybir.AluOpType
PM = mybir.MatmulPerfMode
BFNP = ml_dtypes.bfloat16
F8NP = ml_dtypes.float8_e4m3

B, C, N = 32, 1024, 256
DQ = 64
NCORES = 8
BS = B // NCORES  # samples per core
CCH = C // 128    # chunks of the channel dim
KCH = N // 128    # chunks of the position dim (qk contraction)
EPS = 1e-5
WV_SCALE = 16.0   # keeps fp8 Wv weights in e4m3 normal range
E_SCALE_LN = 2.772588722239781  # ln(16): exp tiles carry a 16x factor
NP = 264          # padded v free size (fp8 DoubleRow AP needs %16 strides)
LAST_RESULTS = None  # BassKernelResults of the most recent run
_NC_CACHE = {}


def _build(bs: int = BS):
    nc = bacc.Bacc("TRN2", target_bir_lowering=False, debug=False)

    xb_d = nc.dram_tensor("xb_in", (bs, 128, CCH, N), BF16, kind="ExternalInput")
    xt_d = nc.dram_tensor("xt_in", (bs, 128, KCH, C), FP8, kind="ExternalInput")
    # fp8 x with two samples interleaved at the n level: [g, p, kc, j, n]
    xb8_d = nc.dram_tensor("xb8_in", (bs // 2, 128, CCH, 2, N), FP8,
                           kind="ExternalInput")
    wqkt_d = nc.dram_tensor("wqkt", (128, KCH, 128), FP8, kind="ExternalInput")
    tqk_d = nc.dram_tensor("tqk", (128, 1), F32, kind="ExternalInput")
    wvt_d = nc.dram_tensor("wvt", (128, CCH, C), FP8, kind="ExternalInput")
    t3_d = nc.dram_tensor("t3", (128, CCH), F32, kind="ExternalInput")
    vcol_d = nc.dram_tensor("vcol", (128, CCH, 2, 2), BF16, kind="ExternalInput")
    out_d = nc.dram_tensor("y_out", (bs, 128, CCH, N), BF16, kind="ExternalOutput")

    with tile.TileContext(nc) as tc:
        with (
            tc.tile_pool(name="consts", bufs=1) as consts,
            tc.tile_pool(name="xbp", bufs=4) as xbp,
            tc.tile_pool(name="xtp", bufs=2) as xtp,
            tc.tile_pool(name="x8p", bufs=2) as x8p,
            tc.tile_pool(name="qkp", bufs=2) as qkp,
            tc.tile_pool(name="qsp", bufs=2) as qsp,
            tc.tile_pool(name="etp", bufs=10) as etp,
            tc.tile_pool(name="vp", bufs=2) as vp,
            tc.tile_pool(name="outp", bufs=2) as outp,
            tc.tile_pool(name="smallp", bufs=8) as smallp,
            tc.tile_pool(name="psbig", bufs=2, space="PSUM") as psbig,
            tc.tile_pool(name="psv", bufs=2, space="PSUM") as psvp,
            tc.tile_pool(name="psbank", bufs=2, space="PSUM") as psbank,
        ):
            # all weights on the HW sync queue, small ones first: wqkt must
            # land before the first qk projection (~4us in), wvt8 before the
            # first v chunk (~7us)
            wqkt = consts.tile([128, KCH, 128], FP8, tag="wqkt")
            nc.sync.dma_start(out=wqkt, in_=wqkt_d[:])
            tqk = consts.tile([128, 1], F32, tag="tqk")
            nc.sync.dma_start(out=tqk, in_=tqk_d[:])
            t3 = consts.tile([128, CCH], F32, tag="t3")
            nc.sync.dma_start(out=t3, in_=t3_d[:])
            vcol4 = consts.tile([128, CCH, 2, 2], BF16, tag="vcol")
            nc.sync.dma_start(out=vcol4, in_=vcol_d[:])

            zero = consts.tile([128, 1], F32, tag="zero")
            nc.vector.memset(zero, 0.0)
            ln16 = consts.tile([128, 1], F32, tag="ln16")
            nc.vector.memset(ln16, E_SCALE_LN)
            # touch the activation table early so the lazy ACT_TABLE_LOAD
            # doesn't delay the first (critical-path) exp
            warm = consts.tile([128, 1], F32, tag="warm")
            nc.scalar.activation(out=warm, in_=zero, func=AF.Exp,
                                 bias=zero[:, 0:1], scale=1.0)
            # dummy-matmul fodder: keeps the PE busy through the initial DMA
            # wait so the HAM clock gate reaches 8/8 before the real work
            zwarm = consts.tile([128, 256], BF16, tag="zwarm")
            nc.vector.memset(zwarm, 0.0)

            def warm_burst(n):
                warm_ps = psbank.tile([128, NP], F32, tag="psbank")
                for _ in range(n):
                    nc.tensor.matmul(
                        warm_ps[:, 0:256],
                        zwarm[:, 0:128],
                        zwarm,
                        start=True,
                        stop=True,
                    )

            xb_sb = [None] * bs
            xt_sb = [None] * bs
            x8_sb = [None] * (bs // 2)
            qk_sb = [None] * bs
            qsw = [None] * bs
            v2_sb = [None] * (bs // 2)
            et = [[None] * (CCH // 2) for _ in range(bs)]
            o_sb = [None] * bs
            wvt8 = None

            def load_wvt8():
                nonlocal wvt8
                wvt8 = consts.tile([128, CCH, C], FP8, tag="wvt8")
                nc.sync.dma_start(out=wvt8, in_=wvt_d[:])

            def load_xt(b):
                xt_sb[b] = xtp.tile([128, KCH, C], FP8, tag="xt",
                                    name=f"xt_sb{b}")
                nc.sync.dma_start(out=xt_sb[b], in_=xt_d[b])

            def load_xb(b):
                xb_sb[b] = xbp.tile([128, CCH, N], BF16, tag="xb",
                                    name=f"xb_sb{b}")
                nc.sync.dma_start(out=xb_sb[b], in_=xb_d[b])

            def load_x8(g):
                x8_sb[g] = x8p.tile([128, CCH, 2, N], FP8, tag="x8",
                                    name=f"x8_{g}")
                nc.sync.dma_start(out=x8_sb[g], in_=xb8_d[g])

            def qk_phase(b):
                # q/k projection: psum rows 0:64 = q, 64:128 = k
                qk_sb[b] = qkp.tile([128, C], BF16, tag="qk", name=f"qk_sb{b}")
                for cb in range(C // 512):
                    qk_ps = psvp.tile([128, 512], F32, tag="psv")
                    nc.tensor.matmul(
                        qk_ps,
                        wqkt,
                        xt_sb[b][:, :, cb * 512:(cb + 1) * 512],
                        start=True,
                        stop=True,
                        perf_mode=PM.DoubleRow,
                    )
                    # bias+relu on the DVE: keeps the latency-critical relu
                    # out of the ACT FIFO (which is busy draining exp tiles)
                    nc.vector.tensor_scalar(
                        out=qk_sb[b][:, cb * 512:(cb + 1) * 512], in0=qk_ps,
                        scalar1=tqk[:, 0:1], scalar2=0.0,
                        op0=ALU.add, op1=ALU.max,
                    )
                # row-tiled sim needs k at partitions 0:63 and a second copy
                # of q at 64:127; the scalar ring is idle here so the copies
                # issue promptly after the relu
                qsw[b] = qsp.tile([128, C], BF16, tag="qsw", name=f"qsw{b}")
                nc.scalar.dma_start(out=qsw[b][0:64, :], in_=qk_sb[b][64:128, :])
                nc.scalar.dma_start(out=qsw[b][64:128, :], in_=qk_sb[b][0:64, :])

            def sim_dp(b, dp):
                # transposed sim + exp for one pair of d-chunks:
                # et[b][dp][dd, j, c] = 16*exp(-sim[c, 2dp+j*128+dd]).
                # two d-chunks run concurrently via PE row tiling (K=64 each).
                d0, d1 = 2 * dp, 2 * dp + 1
                psA = psbig.tile([128, C], F32, tag="psbig")
                psB = psbig.tile([128, C], F32, tag="psbig")
                for cb in range(C // 512):
                    sl = slice(cb * 512, (cb + 1) * 512)
                    nc.tensor.matmul(
                        psA[:, sl],
                        qk_sb[b][0:64, d0 * 128:(d0 + 1) * 128],
                        qsw[b][0:64, sl],
                        start=True, stop=True,
                    )
                    nc.tensor.matmul(
                        psB[:, sl],
                        qsw[b][64:128, d1 * 128:(d1 + 1) * 128],
                        qk_sb[b][64:128, sl],
                        start=True, stop=True,
                    )
                e2 = etp.tile([128, 2, C], FP8, tag="et")
                for j, ps in ((0, psA), (1, psB)):
                    # 16*exp(-sim) on the ACT LUT (bias = ln 16)
                    nc.scalar.activation(
                        out=e2[:, j, :], in_=ps, func=AF.Exp,
                        bias=ln16[:, 0:1], scale=-1.0,
                    )
                et[b][dp] = e2

            def v_chunk(g, m):
                # v output chunk m for samples 2g/2g+1 in fp8 DoubleRow:
                # rhs [p, kcpair, j, n] streams 512 virtual columns per
                # weight load. v carries a WV_SCALE factor; the ones-columns
                # put the same factor into Z so U*(1/Z) comes out as
                # alpha*(aff @ v).
                if m == 0:
                    v2_sb[g] = vp.tile([128, CCH, 2, NP], FP8, tag="v",
                                       name=f"v2_{g}")
                    # Z ones-columns (cols N, N+1): tiny strided DVE copy
                    # (the old SWDGE fill flooded the queue with 8B packets)
                    nc.vector.tensor_copy(
                        out=v2_sb[g][:, :, :, N:N + 2], in_=vcol4)
                psv = psvp.tile([128, 2 * N], F32, tag="psv")
                for kcp in range(CCH // 2):
                    nc.tensor.matmul(
                        psv,
                        wvt8[:, 2 * kcp:2 * kcp + 2, m * 128:(m + 1) * 128],
                        x8_sb[g][:, 2 * kcp:2 * kcp + 2, :, :],
                        start=(kcp == 0),
                        stop=(kcp == CCH // 2 - 1),
                        perf_mode=PM.DoubleRow,
                    )
                nc.vector.tensor_scalar(
                    out=v2_sb[g][:, m, :, 0:N],
                    in0=psv,
                    scalar1=t3[:, m:m + 1],
                    scalar2=0.0,
                    op0=ALU.add,
                    op1=ALU.max,
                )

            def u_chunk(b, m):
                # U = E @ v_ext for output chunk m (col N accumulates
                # (16/alpha)*Z), then out = U * (1/Zcol) + x fused on the DVE
                if m == 0:
                    o_sb[b] = outp.tile([128, CCH, N], BF16, tag="o",
                                        name=f"o_sb{b}")
                u_ps = psbank.tile([128, NP], F32, tag="psbank")
                for dp in range(CCH // 2):
                    nc.tensor.matmul(
                        u_ps,
                        et[b][dp][:, :, m * 128:(m + 1) * 128],
                        v2_sb[b // 2][:, 2 * dp:2 * dp + 2, b % 2, :],
                        start=(dp == 0),
                        stop=(dp == CCH // 2 - 1),
                        perf_mode=PM.DoubleRow,
                    )
                rz = smallp.tile([128, 1], F32, tag="rz")
                nc.vector.reciprocal_approx_fast(out=rz, in_=u_ps[:, N:N + 1])
                nc.vector.scalar_tensor_tensor(
                    out=o_sb[b][:, m, :],
                    in0=u_ps[:, 0:N],
                    scalar=rz[:, 0:1],
                    in1=xb_sb[b][:, m, :],
                    op0=ALU.mult,
                    op1=ALU.add,
                )
                if m % 2 == 1:
                    # stream the result out in 2-chunk pieces so the
                    # store overlaps the remaining compute
                    nc.sync.dma_start(
                        out=out_d[b, :, m - 1:m + 1, :],
                        in_=o_sb[b][:, m - 1:m + 1, :],
                    )

            # ---- software-pipelined program order ----
            # loads for samples 0/1 first; wvt8 right after xt0 so the
            # first v chunks (~7us) have it
            load_xt(0)
            load_wvt8()
            load_x8(0)
            load_xb(0)
            load_xt(1)
            load_xb(1)

            warm_burst(12)
            qk_phase(0)
            # bridge the qk->qsw latency (~3us: DVE relu + scalar-q copies)
            warm_burst(4)
            v_chunk(0, 0)
            v_chunk(0, 1)

            sim_dp(0, 0); v_chunk(0, 2)
            sim_dp(0, 1); v_chunk(0, 3)
            qk_phase(1)
            sim_dp(0, 2); v_chunk(0, 4); v_chunk(0, 5)
            sim_dp(0, 3); v_chunk(0, 6); v_chunk(0, 7)

            sim_dp(1, 0); u_chunk(0, 0); u_chunk(0, 1)
            load_xt(2); load_xb(2); load_x8(1)
            sim_dp(1, 1); u_chunk(0, 2)
            qk_phase(2)
            sim_dp(1, 2); u_chunk(0, 3); u_chunk(0, 4)
            sim_dp(1, 3); u_chunk(0, 5); u_chunk(0, 6)

            sim_dp(2, 0); u_chunk(0, 7); u_chunk(1, 0)
            load_xt(3); load_xb(3)
            sim_dp(2, 1); u_chunk(1, 1); v_chunk(1, 0)
            qk_phase(3)
            sim_dp(2, 2); u_chunk(1, 2); u_chunk(1, 3); v_chunk(1, 1)
            sim_dp(2, 3); u_chunk(1, 4); u_chunk(1, 5); v_chunk(1, 2)

            sim_dp(3, 0); u_chunk(1, 6); u_chunk(1, 7); v_chunk(1, 3)
            sim_dp(3, 1); v_chunk(1, 4); v_chunk(1, 5)
            sim_dp(3, 2); v_chunk(1, 6); v_chunk(1, 7)
            sim_dp(3, 3); u_chunk(2, 0); u_chunk(2, 1)

            for m in range(2, CCH):
                u_chunk(2, m)
            for m in range(CCH):
                u_chunk(3, m)

    nc.compile()
    return nc


def _prep_host(x, Wq, Wk, Wv, bn1_g, bn1_b, bn1_m, bn1_v,
               bn2_g, bn2_b, bn2_m, bn2_v, bn3_g, bn3_b, bn3_m, bn3_v):
    f = np.float32
    s1 = (bn1_g / np.sqrt(bn1_v + EPS)).astype(f)
    t1 = (bn1_b - s1 * bn1_m).astype(f)
    s2 = (bn2_g / np.sqrt(bn2_v + EPS)).astype(f)
    t2 = (bn2_b - s2 * bn2_m).astype(f)
    s3 = (bn3_g / np.sqrt(bn3_v + EPS)).astype(f)
    t3 = ((bn3_b - s3 * bn3_m) * WV_SCALE).astype(f)

    wqk = np.concatenate([Wq * s1[:, None], Wk * s2[:, None]], axis=0).astype(f)
    # lhsT layout [p(=n local), kc, o], fp8
    wqkt = np.ascontiguousarray(
        wqk.T.reshape(KCH, 128, 128).transpose(1, 0, 2)).astype(F8NP)
    tqk = np.concatenate([t1, t2]).reshape(128, 1).astype(f)

    wv2 = (Wv * (s3 * WV_SCALE)[:, None]).astype(f)
    # lhsT layout [p(=ci local), kc, co], fp8
    wvt8 = np.ascontiguousarray(
        wv2.T.reshape(CCH, 128, C).transpose(1, 0, 2)).astype(F8NP)
    t3r = np.ascontiguousarray(t3.reshape(CCH, 128).T)

    x = np.asarray(x, dtype=f)
    # [b, p(=c local), kc, n]
    xq = x.reshape(B, CCH, 128, N).transpose(0, 2, 1, 3)
    xb = np.ascontiguousarray(xq).astype(BFNP)
    # [g, p, kc, j, n] fp8 pairs for the DoubleRow v matmul
    xb8 = np.ascontiguousarray(
        xq.reshape(B // 2, 2, 128, CCH, N).transpose(0, 2, 3, 1, 4)
    ).astype(F8NP)
    # [b, p(=n local), kc, c], fp8 for the DoubleRow qk matmul
    xt = np.ascontiguousarray(
        x.transpose(0, 2, 1).reshape(B, KCH, 128, C).transpose(0, 2, 1, 3)
    ).astype(F8NP)
    return xb, xb8, xt, wqkt, tqk, wvt8, t3r


def kernel(x, Wq, Wk, Wv,
           bn1_g, bn1_b, bn1_m, bn1_v,
           bn2_g, bn2_b, bn2_m, bn2_v,
           bn3_g, bn3_b, bn3_m, bn3_v,
           alpha):
    global LAST_RESULTS
    args = [np.asarray(a, dtype=np.float32) for a in (
        x, Wq, Wk, Wv, bn1_g, bn1_b, bn1_m, bn1_v,
        bn2_g, bn2_b, bn2_m, bn2_v, bn3_g, bn3_b, bn3_m, bn3_v)]
    alpha_val = float(np.asarray(alpha).reshape(-1)[0])
    if alpha_val == 0.0:
        return np.asarray(x, dtype=np.float32).copy()

    xb, xb8, xt, wqkt, tqk, wvt8, t3r = _prep_host(*args)
    # the Z column accumulates (WV_SCALE/alpha)*Z, so U*(1/Zcol) yields
    # alpha*(aff @ v) with both the fp8 weight scale and alpha folded in
    vcol = np.full((128, CCH, 2, 2), WV_SCALE / alpha_val, dtype=BFNP)

    if "nc" not in _NC_CACHE:
        _NC_CACHE["nc"] = _build()
    nc = _NC_CACHE["nc"]

    in_maps = []
    for cid in range(NCORES):
        sl = slice(cid * BS, (cid + 1) * BS)
        slg = slice(cid * (BS // 2), (cid + 1) * (BS // 2))
        in_maps.append({
            "xb_in": np.ascontiguousarray(xb[sl]),
            "xb8_in": np.ascontiguousarray(xb8[slg]),
            "xt_in": np.ascontiguousarray(xt[sl]),
            "wqkt": wqkt,
            "tqk": tqk,
            "wvt": wvt8,
            "t3": t3r,
            "vcol": vcol,
        })

    trace = bool(int(os.environ.get("KERNEL_TRACE", "0")))
    tmpdir = os.environ.get("KERNEL_TRACE_DIR") or None
    res = run_bass_kernel_spmd(
        nc, in_maps, core_ids=list(range(NCORES)), trace=trace, tmpdir=tmpdir
    )
    LAST_RESULTS = res

    y = np.concatenate(
        [np.asarray(res.results[cid]["y_out"], dtype=np.float32)
         for cid in range(NCORES)], axis=0)
    y = y.transpose(0, 2, 1, 3).reshape(B, C, N)
    return np.ascontiguousarray(y)


# revision 8
# speedup vs baseline: 1.1728x; 1.0365x over previous
"""Trainium2 Bass kernel for the CAA (channel-affinity attention) module.

Reference computation per sample b (C=1024 channels, N=256 positions):
    x_hat = x^T                              (N, C)
    q = relu(BN1(Wq @ x_hat))                (64, C)
    k = relu(BN2(Wk @ x_hat))                (64, C)
    sim[c, d] = sum_o k[o, c] * q[o, d]      (C, C)
    aff = softmax(rowmax(sim) - sim, axis d) == softmax(-sim, axis d)
    v = relu(BN3(Wv @ x))                    (C, N)
    out = alpha * (aff @ v) + x              (C, N)

Device-side strategy (pure data parallel, 4 samples per core x 8 cores):
  * BN folded into weights/bias on the host.
  * sim is computed TRANSPOSED (d on partitions, c on free) so the exp(-sim)
    tiles feed the aff @ v contraction directly as matmul lhsT.
  * sim uses 2-way PE row tiling: the K=64 contraction only fills half the
    array, so two d-chunks run concurrently in rows 0-63 / 64-127 (needs k
    copied to partitions 0-63 and q to 64-127 -- one SBUF-SBUF DMA each).
  * the qk projection, v = relu(Wv'x + t3), and U = E @ v_ext all run in
    fp8 (e4m3) with DoubleRow perf mode, which packs two contraction rows
    per PE cell (2x throughput). v packs two samples side by side in the
    moving operand so each matmul streams 512 virtual columns per weight
    load; U pairs adjacent d-chunks via [p, 2, free] access patterns.
  * exp tiles are 16*exp(-sim) in fp8 (ACT LUT, bias=ln16): the 16x keeps
    e4m3 from underflowing whole softmax rows (row-min sim <= ~10), and
    the same factor enters Z through the 16/alpha ones-columns, so it
    cancels in U * (1/Zcol). Wv carries a further 16x for the same reason.
  * epilogues: v bias+relu and the fused U*(1/Z)+x reside on the DVE
    (reciprocal_approx_fast for 1/Z); exp owns the ACT; the PE clock is
    kept warm through the initial DMA wait by dummy matmuls.
  * the residual add reads the bf16 copy of x; output is stored bf16.

Scheduling (v2): all weights load FIRST on the HW sync queue (the old
SWDGE path delivered wqkt at ~17us and stalled the first projection), and
the per-sample phases are software-pipelined at chunk granularity: sim
dp-chunks are interleaved with u/v m-chunks in program order so the PE
engine queue always has >~2.3us of independent work between consecutive
sim chunks (psbig has 2 PSUM bufs; sim chunk t+1 must wait for ACT to
drain chunk t's exps -- FIFO queues stall everything behind it otherwise).
"""

import os
import sys

import numpy as np

_REPO = "/opt/trn_rl_repo"
if _REPO not in sys.path:
    sys.path.insert(0, _REPO)

import ml_dtypes  # noqa: E402

import concourse.bacc as bacc  # noqa: E402
import concourse.tile as tile  # noqa: E402
from concourse import mybir  # noqa: E402
from concourse.bass_utils import run_bass_kernel_spmd  # noqa: E402

F32 = mybir.dt.float32
BF16 = mybir.dt.bfloat16
FP8 = mybir.dt.float8e4
AF = mybir.ActivationFunctionType
ALU = mybir.AluOpType
PM = mybir.MatmulPerfMode
BFNP = ml_dtypes.bfloat16
F8NP = ml_dtypes.float8_e4m3

B, C, N = 32, 1024, 256
DQ = 64
NCORES = 8
BS = B // NCORES  # samples per core
CCH = C // 128    # chunks of the channel dim
KCH = N // 128    # chunks of the position dim (qk contraction)
EPS = 1e-5
WV_SCALE = 16.0   # keeps fp8 Wv weights in e4m3 normal range
E_SCALE_LN = 2.772588722239781  # ln(16): exp tiles carry a 16x factor
NP = 264          # padded v free size (fp8 DoubleRow AP needs %16 strides)
LAST_RESULTS = None  # BassKernelResults of the most recent run
_NC_CACHE = {}


def _build(bs: int = BS):
    nc = bacc.Bacc("TRN2", target_bir_lowering=False, debug=False)

    xb_d = nc.dram_tensor("xb_in", (bs, 128, CCH, N), BF16, kind="ExternalInput")
    xt_d = nc.dram_tensor("xt_in", (bs, 128, KCH, C), FP8, kind="ExternalInput")
    # fp8 x with two samples interleaved at the n level: [g, p, kc, j, n]
    xb8_d = nc.dram_tensor("xb8_in", (bs // 2, 128, CCH, 2, N), FP8,
                           kind="ExternalInput")
    wqkt_d = nc.dram_tensor("wqkt", (128, KCH, 128), FP8, kind="ExternalInput")
    tqk_d = nc.dram_tensor("tqk", (128, 1), F32, kind="ExternalInput")
    wvt_d = nc.dram_tensor("wvt", (128, CCH, C), FP8, kind="ExternalInput")
    t3_d = nc.dram_tensor("t3", (128, CCH), F32, kind="ExternalInput")
    vcol_d = nc.dram_tensor("vcol", (128, CCH, 2, 2), BF16, kind="ExternalInput")
    out_d = nc.dram_tensor("y_out", (bs, 128, CCH, N), BF16, kind="ExternalOutput")

    with tile.TileContext(nc) as tc:
        with (
            tc.tile_pool(name="consts", bufs=1) as consts,
            tc.tile_pool(name="xbp", bufs=4) as xbp,
            tc.tile_pool(name="xtp", bufs=2) as xtp,
            tc.tile_pool(name="x8p", bufs=2) as x8p,
            tc.tile_pool(name="qkp", bufs=2) as qkp,
            tc.tile_pool(name="qsp", bufs=2) as qsp,
            tc.tile_pool(name="etp", bufs=10) as etp,
            tc.tile_pool(name="vp", bufs=2) as vp,
            tc.tile_pool(name="outp", bufs=2) as outp,
            tc.tile_pool(name="smallp", bufs=8) as smallp,
            tc.tile_pool(name="psbig", bufs=2, space="PSUM") as psbig,
            tc.tile_pool(name="psv", bufs=2, space="PSUM") as psvp,
            tc.tile_pool(name="psbank", bufs=2, space="PSUM") as psbank,
        ):
            # each DMA trigger occupies its queue ~0.65us, so the head loads
            # are spread across queues: sync carries wqkt + the big x
            # tensors (in need-order), the scalar queue the tiny biases
            wqkt = consts.tile([128, KCH, 128], FP8, tag="wqkt")
            nc.sync.dma_start(out=wqkt, in_=wqkt_d[:])
            tqk = consts.tile([128, 1], F32, tag="tqk")
            nc.scalar.dma_start(out=tqk, in_=tqk_d[:])
            t3 = consts.tile([128, CCH], F32, tag="t3")
            nc.scalar.dma_start(out=t3, in_=t3_d[:])
            vcol4 = consts.tile([128, CCH, 2, 2], BF16, tag="vcol")
            nc.scalar.dma_start(out=vcol4, in_=vcol_d[:])

            zero = consts.tile([128, 1], F32, tag="zero")
            nc.gpsimd.memset(zero, 0.0)
            ln16 = consts.tile([128, 1], F32, tag="ln16")
            nc.gpsimd.memset(ln16, E_SCALE_LN)
            # touch the activation table early so the lazy ACT_TABLE_LOAD
            # doesn't delay the first (critical-path) exp
            warm = consts.tile([128, 1], F32, tag="warm")
            nc.scalar.activation(out=warm, in_=zero, func=AF.Exp,
                                 bias=zero[:, 0:1], scale=1.0)
            # dummy-matmul fodder: keeps the PE busy through the initial DMA
            # wait so the HAM clock gate reaches 8/8 before the real work
            # (memset on gpsimd: its icache loads early and the DVE queue
            # stays clear for the first qk relu)
            zwarm = consts.tile([128, 256], BF16, tag="zwarm")
            nc.gpsimd.memset(zwarm, 0.0)

            def warm_burst(n):
                warm_ps = psbank.tile([128, NP], F32, tag="psbank")
                for _ in range(n):
                    nc.tensor.matmul(
                        warm_ps[:, 0:256],
                        zwarm[:, 0:128],
                        zwarm,
                        start=True,
                        stop=True,
                    )

            xb_sb = [None] * bs
            xt_sb = [None] * bs
            x8_sb = [None] * (bs // 2)
            qk_sb = [None] * bs
            qsw = [None] * bs
            v2_sb = [None] * (bs // 2)
            et = [[None] * (CCH // 2) for _ in range(bs)]
            o_sb = [None] * bs
            wvt8 = None

            def load_wvt8():
                nonlocal wvt8
                wvt8 = consts.tile([128, CCH, C], FP8, tag="wvt8")
                nc.sync.dma_start(out=wvt8, in_=wvt_d[:])

            def load_xt(b):
                xt_sb[b] = xtp.tile([128, KCH, C], FP8, tag="xt",
                                    name=f"xt_sb{b}")
                nc.sync.dma_start(out=xt_sb[b], in_=xt_d[b])

            def load_xb(b):
                xb_sb[b] = xbp.tile([128, CCH, N], BF16, tag="xb",
                                    name=f"xb_sb{b}")
                nc.sync.dma_start(out=xb_sb[b], in_=xb_d[b])

            def load_x8(g):
                x8_sb[g] = x8p.tile([128, CCH, 2, N], FP8, tag="x8",
                                    name=f"x8_{g}")
                nc.sync.dma_start(out=x8_sb[g], in_=xb8_d[g])

            def qk_phase(b):
                # q/k projection: psum rows 0:64 = q, 64:128 = k
                qk_sb[b] = qkp.tile([128, C], BF16, tag="qk", name=f"qk_sb{b}")
                # row-tiled sim needs k at partitions 0:63 and a second copy
                # of q at 64:127; the copies ride the otherwise-idle gpsimd
                # queue (the scalar queue is saturated with exp tiles) and
                # are split per 512-half so sim dp=0/1 can start early
                qsw[b] = qsp.tile([128, C], BF16, tag="qsw", name=f"qsw{b}")
                for cb in range(C // 512):
                    sl = slice(cb * 512, (cb + 1) * 512)
                    qk_ps = psvp.tile([128, 512], F32, tag="psv")
                    nc.tensor.matmul(
                        qk_ps,
                        wqkt,
                        xt_sb[b][:, :, sl],
                        start=True,
                        stop=True,
                        perf_mode=PM.DoubleRow,
                    )
                    # bias+relu on the DVE: keeps the latency-critical relu
                    # out of the ACT FIFO (which is busy draining exp tiles)
                    nc.vector.tensor_scalar(
                        out=qk_sb[b][:, sl], in0=qk_ps,
                        scalar1=tqk[:, 0:1], scalar2=0.0,
                        op0=ALU.add, op1=ALU.max,
                    )
                    nc.gpsimd.dma_start(out=qsw[b][0:64, sl],
                                        in_=qk_sb[b][64:128, sl])
                    nc.gpsimd.dma_start(out=qsw[b][64:128, sl],
                                        in_=qk_sb[b][0:64, sl])

            def sim_dp(b, dp):
                # transposed sim + exp for one pair of d-chunks:
                # et[b][dp][dd, j, c] = 16*exp(-sim[c, 2dp+j*128+dd]).
                # two d-chunks run concurrently via PE row tiling (K=64 each).
                d0, d1 = 2 * dp, 2 * dp + 1
                psA = psbig.tile([128, C], F32, tag="psbig")
                psB = psbig.tile([128, C], F32, tag="psbig")
                for cb in range(C // 512):
                    sl = slice(cb * 512, (cb + 1) * 512)
                    nc.tensor.matmul(
                        psA[:, sl],
                        qk_sb[b][0:64, d0 * 128:(d0 + 1) * 128],
                        qsw[b][0:64, sl],
                        start=True, stop=True,
                    )
                    nc.tensor.matmul(
                        psB[:, sl],
                        qsw[b][64:128, d1 * 128:(d1 + 1) * 128],
                        qk_sb[b][64:128, sl],
                        start=True, stop=True,
                    )
                e2 = etp.tile([128, 2, C], FP8, tag="et")
                for j, ps in ((0, psA), (1, psB)):
                    # 16*exp(-sim) on the ACT LUT (bias = ln 16)
                    nc.scalar.activation(
                        out=e2[:, j, :], in_=ps, func=AF.Exp,
                        bias=ln16[:, 0:1], scale=-1.0,
                    )
                et[b][dp] = e2

            def v_chunk(g, m):
                # v output chunk m for samples 2g/2g+1 in fp8 DoubleRow:
                # rhs [p, kcpair, j, n] streams 512 virtual columns per
                # weight load. v carries a WV_SCALE factor; the ones-columns
                # put the same factor into Z so U*(1/Z) comes out as
                # alpha*(aff @ v).
                if m == 0:
                    v2_sb[g] = vp.tile([128, CCH, 2, NP], FP8, tag="v",
                                       name=f"v2_{g}")
                    # Z ones-columns (cols N, N+1): tiny strided DVE copy
                    # (the old SWDGE fill flooded the queue with 8B packets)
                    nc.vector.tensor_copy(
                        out=v2_sb[g][:, :, :, N:N + 2], in_=vcol4)
                psv = psvp.tile([128, 2 * N], F32, tag="psv")
                for kcp in range(CCH // 2):
                    nc.tensor.matmul(
                        psv,
                        wvt8[:, 2 * kcp:2 * kcp + 2, m * 128:(m + 1) * 128],
                        x8_sb[g][:, 2 * kcp:2 * kcp + 2, :, :],
                        start=(kcp == 0),
                        stop=(kcp == CCH // 2 - 1),
                        perf_mode=PM.DoubleRow,
                    )
                nc.vector.tensor_scalar(
                    out=v2_sb[g][:, m, :, 0:N],
                    in0=psv,
                    scalar1=t3[:, m:m + 1],
                    scalar2=0.0,
                    op0=ALU.add,
                    op1=ALU.max,
                )

            def u_chunk(b, m):
                # U = E @ v_ext for output chunk m (col N accumulates
                # (16/alpha)*Z), then out = U * (1/Zcol) + x fused on the DVE
                if m == 0:
                    o_sb[b] = outp.tile([128, CCH, N], BF16, tag="o",
                                        name=f"o_sb{b}")
                u_ps = psbank.tile([128, NP], F32, tag="psbank")
                for dp in range(CCH // 2):
                    nc.tensor.matmul(
                        u_ps,
                        et[b][dp][:, :, m * 128:(m + 1) * 128],
                        v2_sb[b // 2][:, 2 * dp:2 * dp + 2, b % 2, :],
                        start=(dp == 0),
                        stop=(dp == CCH // 2 - 1),
                        perf_mode=PM.DoubleRow,
                    )
                rz = smallp.tile([128, 1], F32, tag="rz")
                nc.vector.reciprocal_approx_fast(out=rz, in_=u_ps[:, N:N + 1])
                nc.vector.scalar_tensor_tensor(
                    out=o_sb[b][:, m, :],
                    in0=u_ps[:, 0:N],
                    scalar=rz[:, 0:1],
                    in1=xb_sb[b][:, m, :],
                    op0=ALU.mult,
                    op1=ALU.add,
                )
                if m % 2 == 1:
                    # stream the result out in 2-chunk pieces so the
                    # store overlaps the remaining compute
                    nc.sync.dma_start(
                        out=out_d[b, :, m - 1:m + 1, :],
                        in_=o_sb[b][:, m - 1:m + 1, :],
                    )

            # ---- software-pipelined program order ----
            # sims run at the ACT drain cadence (~2.2us per dp pair, gated
            # by the 2-buf psbig pool); every inter-sim slot carries ~2
            # independent u/v chunks so the PE never idles at the psbig
            # wait, and exp(3,3) -- which gates the serial u3 tail --
            # completes as early as the ACT allows.
            load_xt(0)
            load_x8(0)
            load_wvt8()
            load_xb(0)
            load_xt(1)
            load_xb(1)
            load_xt(2)

            warm_burst(8)
            qk_phase(0)
            warm_burst(4)
            v_chunk(0, 0)
            sim_dp(0, 0); v_chunk(0, 1)
            sim_dp(0, 1); v_chunk(0, 2); qk_phase(1)
            sim_dp(0, 2); v_chunk(0, 3); v_chunk(0, 4)
            sim_dp(0, 3); v_chunk(0, 5); v_chunk(0, 6)

            load_xb(2); load_x8(1); load_xt(3); load_xb(3)
            sim_dp(1, 0); v_chunk(0, 7); qk_phase(2)
            sim_dp(1, 1); u_chunk(0, 0); u_chunk(0, 1)
            sim_dp(1, 2); u_chunk(0, 2); u_chunk(0, 3)
            sim_dp(1, 3); u_chunk(0, 4); qk_phase(3)

            sim_dp(2, 0); u_chunk(0, 5); u_chunk(0, 6)
            sim_dp(2, 1); u_chunk(0, 7); v_chunk(1, 0)
            sim_dp(2, 2); v_chunk(1, 1); v_chunk(1, 2)
            sim_dp(2, 3); v_chunk(1, 3); v_chunk(1, 4)

            sim_dp(3, 0); v_chunk(1, 5); v_chunk(1, 6)
            sim_dp(3, 1); v_chunk(1, 7); u_chunk(1, 0)
            sim_dp(3, 2); u_chunk(1, 1); u_chunk(1, 2)
            sim_dp(3, 3); u_chunk(1, 3); u_chunk(1, 4)

            for m in range(5, CCH):
                u_chunk(1, m)
            for m in range(CCH):
                u_chunk(2, m)
            for m in range(CCH):
                u_chunk(3, m)

    nc.compile()
    return nc


def _prep_host(x, Wq, Wk, Wv, bn1_g, bn1_b, bn1_m, bn1_v,
               bn2_g, bn2_b, bn2_m, bn2_v, bn3_g, bn3_b, bn3_m, bn3_v):
    f = np.float32
    s1 = (bn1_g / np.sqrt(bn1_v + EPS)).astype(f)
    t1 = (bn1_b - s1 * bn1_m).astype(f)
    s2 = (bn2_g / np.sqrt(bn2_v + EPS)).astype(f)
    t2 = (bn2_b - s2 * bn2_m).astype(f)
    s3 = (bn3_g / np.sqrt(bn3_v + EPS)).astype(f)
    t3 = ((bn3_b - s3 * bn3_m) * WV_SCALE).astype(f)

    wqk = np.concatenate([Wq * s1[:, None], Wk * s2[:, None]], axis=0).astype(f)
    # lhsT layout [p(=n local), kc, o], fp8
    wqkt = np.ascontiguousarray(
        wqk.T.reshape(KCH, 128, 128).transpose(1, 0, 2)).astype(F8NP)
    tqk = np.concatenate([t1, t2]).reshape(128, 1).astype(f)

    wv2 = (Wv * (s3 * WV_SCALE)[:, None]).astype(f)
    # lhsT layout [p(=ci local), kc, co], fp8
    wvt8 = np.ascontiguousarray(
        wv2.T.reshape(CCH, 128, C).transpose(1, 0, 2)).astype(F8NP)
    t3r = np.ascontiguousarray(t3.reshape(CCH, 128).T)

    x = np.asarray(x, dtype=f)
    # [b, p(=c local), kc, n]
    xq = x.reshape(B, CCH, 128, N).transpose(0, 2, 1, 3)
    xb = np.ascontiguousarray(xq).astype(BFNP)
    # [g, p, kc, j, n] fp8 pairs for the DoubleRow v matmul
    xb8 = np.ascontiguousarray(
        xq.reshape(B // 2, 2, 128, CCH, N).transpose(0, 2, 3, 1, 4)
    ).astype(F8NP)
    # [b, p(=n local), kc, c], fp8 for the DoubleRow qk matmul
    xt = np.ascontiguousarray(
        x.transpose(0, 2, 1).reshape(B, KCH, 128, C).transpose(0, 2, 1, 3)
    ).astype(F8NP)
    return xb, xb8, xt, wqkt, tqk, wvt8, t3r


def kernel(x, Wq, Wk, Wv,
           bn1_g, bn1_b, bn1_m, bn1_v,
           bn2_g, bn2_b, bn2_m, bn2_v,
           bn3_g, bn3_b, bn3_m, bn3_v,
           alpha):
    global LAST_RESULTS
    args = [np.asarray(a, dtype=np.float32) for a in (
        x, Wq, Wk, Wv, bn1_g, bn1_b, bn1_m, bn1_v,
        bn2_g, bn2_b, bn2_m, bn2_v, bn3_g, bn3_b, bn3_m, bn3_v)]
    alpha_val = float(np.asarray(alpha).reshape(-1)[0])
    if alpha_val == 0.0:
        return np.asarray(x, dtype=np.float32).copy()

    xb, xb8, xt, wqkt, tqk, wvt8, t3r = _prep_host(*args)
    # the Z column accumulates (WV_SCALE/alpha)*Z, so U*(1/Zcol) yields
    # alpha*(aff @ v) with both the fp8 weight scale and alpha folded in
    vcol = np.full((128, CCH, 2, 2), WV_SCALE / alpha_val, dtype=BFNP)

    if "nc" not in _NC_CACHE:
        _NC_CACHE["nc"] = _build()
    nc = _NC_CACHE["nc"]

    in_maps = []
    for cid in range(NCORES):
        sl = slice(cid * BS, (cid + 1) * BS)
        slg = slice(cid * (BS // 2), (cid + 1) * (BS // 2))
        in_maps.append({
            "xb_in": np.ascontiguousarray(xb[sl]),
            "xb8_in": np.ascontiguousarray(xb8[slg]),
            "xt_in": np.ascontiguousarray(xt[sl]),
            "wqkt": wqkt,
            "tqk": tqk,
            "wvt": wvt8,
            "t3": t3r,
            "vcol": vcol,
        })

    trace = bool(int(os.environ.get("KERNEL_TRACE", "0")))
    tmpdir = os.environ.get("KERNEL_TRACE_DIR") or None
    res = run_bass_kernel_spmd(
        nc, in_maps, core_ids=list(range(NCORES)), trace=trace, tmpdir=tmpdir
    )
    LAST_RESULTS = res

    y = np.concatenate(
        [np.asarray(res.results[cid]["y_out"], dtype=np.float32)
         for cid in range(NCORES)], axis=0)
    y = y.transpose(0, 2, 1, 3).reshape(B, C, N)
    return np.ascontiguousarray(y)


# revision 18
# speedup vs baseline: 1.3053x; 1.1130x over previous
"""Trainium2 Bass kernel for the CAA (channel-affinity attention) module.

Reference computation per sample b (C=1024 channels, N=256 positions):
    x_hat = x^T                              (N, C)
    q = relu(BN1(Wq @ x_hat))                (64, C)
    k = relu(BN2(Wk @ x_hat))                (64, C)
    sim[c, d] = sum_o k[o, c] * q[o, d]      (C, C)
    aff = softmax(rowmax(sim) - sim, axis d) == softmax(-sim, axis d)
    v = relu(BN3(Wv @ x))                    (C, N)
    out = alpha * (aff @ v) + x              (C, N)

Device-side strategy (pure data parallel, 4 samples per core x 8 cores):
  * BN folded into weights/bias on the host.
  * sim is computed TRANSPOSED (d on partitions, c on free) so the exp(-sim)
    tiles feed the aff @ v contraction directly as matmul lhsT.
  * sim uses 2-way PE row tiling: the K=64 contraction only fills half the
    array, so two d-chunks run concurrently in rows 0-63 / 64-127 (needs k
    copied to partitions 0-63 and q to 64-127 -- one SBUF-SBUF DMA each).
  * the qk projection, v = relu(Wv'x + t3), and U = E @ v_ext all run in
    fp8 (e4m3) with DoubleRow perf mode, which packs two contraction rows
    per PE cell (2x throughput). v packs two samples side by side in the
    moving operand so each matmul streams 512 virtual columns per weight
    load; U pairs adjacent d-chunks via [p, 2, free] access patterns.
  * exp tiles are 16*exp(-sim) in fp8 (ACT LUT, bias=ln16): the 16x keeps
    e4m3 from underflowing whole softmax rows (row-min sim <= ~10), and
    the same factor enters Z through the 16/alpha ones-columns, so it
    cancels in U * (1/Zcol). Wv carries a further 16x for the same reason.
  * epilogues: v bias+relu and the fused U*(1/Z)+x reside on the DVE
    (reciprocal_approx_fast for 1/Z); exp owns the ACT; the PE clock is
    kept warm through the initial DMA wait by dummy matmuls.
  * the residual add reads the bf16 copy of x; output is stored bf16.

Scheduling (v2): all weights load FIRST on the HW sync queue (the old
SWDGE path delivered wqkt at ~17us and stalled the first projection), and
the per-sample phases are software-pipelined at chunk granularity: sim
dp-chunks are interleaved with u/v m-chunks in program order so the PE
engine queue always has >~2.3us of independent work between consecutive
sim chunks (psbig has 2 PSUM bufs; sim chunk t+1 must wait for ACT to
drain chunk t's exps -- FIFO queues stall everything behind it otherwise).
"""

import os
import sys

import numpy as np

_REPO = "/opt/trn_rl_repo"
if _REPO not in sys.path:
    sys.path.insert(0, _REPO)

import ml_dtypes  # noqa: E402

import concourse.bacc as bacc  # noqa: E402
import concourse.tile as tile  # noqa: E402
from concourse import mybir  # noqa: E402
from concourse.bass_utils import run_bass_kernel_spmd  # noqa: E402

F32 = mybir.dt.float32
BF16 = mybir.dt.bfloat16
FP8 = mybir.dt.float8e4
AF = mybir.ActivationFunctionType
ALU = mybir.AluOpType
PM = mybir.MatmulPerfMode
BFNP = ml_dtypes.bfloat16
F8NP = ml_dtypes.float8_e4m3

B, C, N = 32, 1024, 256
DQ = 64
NCORES = 8
BS = B // NCORES  # samples per core
CCH = C // 128    # chunks of the channel dim
KCH = N // 128    # chunks of the position dim (qk contraction)
EPS = 1e-5
WV_SCALE = 16.0   # keeps fp8 Wv weights in e4m3 normal range
E_SCALE_LN = 2.772588722239781  # ln(16): exp tiles carry a 16x factor
NP = 264          # padded v free size (fp8 DoubleRow AP needs %16 strides)
LAST_RESULTS = None  # BassKernelResults of the most recent run
_NC_CACHE = {}


def _build(bs: int = BS):
    nc = bacc.Bacc("TRN2", target_bir_lowering=False, debug=False)

    xb_d = nc.dram_tensor("xb_in", (bs, 128, CCH, N), BF16, kind="ExternalInput")
    xt_d = nc.dram_tensor("xt_in", (bs, 128, KCH, C), FP8, kind="ExternalInput")
    # fp8 x with two samples interleaved at the n level: [g, p, kc, j, n]
    xb8_d = nc.dram_tensor("xb8_in", (bs // 2, 128, CCH, 2, N), FP8,
                           kind="ExternalInput")
    # [p, v, kc, o]: v=0 normal ([q; k] rows), v=1 swapped ([k; q] rows) --
    # the swapped projection writes sample 0's qsw layout directly
    wqkt_d = nc.dram_tensor("wqkt", (128, 2, KCH, 128), FP8, kind="ExternalInput")
    tqk_d = nc.dram_tensor("tqk", (128, 2), F32, kind="ExternalInput")
    wvt_d = nc.dram_tensor("wvt", (128, CCH, C), FP8, kind="ExternalInput")
    t3_d = nc.dram_tensor("t3", (128, CCH), F32, kind="ExternalInput")
    vcol_d = nc.dram_tensor("vcol", (128, CCH, 2, 2), BF16, kind="ExternalInput")
    out_d = nc.dram_tensor("y_out", (bs, 128, CCH, N), BF16, kind="ExternalOutput")

    with tile.TileContext(nc) as tc:
        with (
            tc.tile_pool(name="consts", bufs=1) as consts,
            tc.tile_pool(name="xbp", bufs=4) as xbp,
            tc.tile_pool(name="xtp", bufs=2) as xtp,
            tc.tile_pool(name="x8p", bufs=2) as x8p,
            tc.tile_pool(name="qkp", bufs=2) as qkp,
            tc.tile_pool(name="qsp", bufs=2) as qsp,
            tc.tile_pool(name="etp", bufs=10) as etp,
            tc.tile_pool(name="vp", bufs=2) as vp,
            tc.tile_pool(name="outp", bufs=2) as outp,
            tc.tile_pool(name="smallp", bufs=8) as smallp,
            # psbig holds 3 sim psum pairs so the sim matmuls never gate on
            # the ACT exp drain (with 2 bufs the exp->matmul->exp round trip
            # added ~1us per dp pair); qk/v/u share one single-bank pool
            tc.tile_pool(name="psbig", bufs=3, space="PSUM") as psbig,
            tc.tile_pool(name="pswk", bufs=2, space="PSUM") as pswk,
        ):
            # each DMA trigger occupies its queue ~0.65us, so the head loads
            # are spread across queues: sync carries wqkt + the big x
            # tensors (in need-order), the scalar queue the tiny biases
            wqkt = consts.tile([128, 2, KCH, 128], FP8, tag="wqkt")
            nc.sync.dma_start(out=wqkt, in_=wqkt_d[:])
            tqk = consts.tile([128, 2], F32, tag="tqk")
            nc.scalar.dma_start(out=tqk, in_=tqk_d[:])
            t3 = consts.tile([128, CCH], F32, tag="t3")
            nc.scalar.dma_start(out=t3, in_=t3_d[:])
            vcol4 = consts.tile([128, CCH, 2, 2], BF16, tag="vcol")
            nc.scalar.dma_start(out=vcol4, in_=vcol_d[:])

            zero = consts.tile([128, 1], F32, tag="zero")
            nc.gpsimd.memset(zero, 0.0)
            ln16 = consts.tile([128, 1], F32, tag="ln16")
            nc.gpsimd.memset(ln16, E_SCALE_LN)
            # touch the activation table early so the lazy ACT_TABLE_LOAD
            # doesn't delay the first (critical-path) exp
            warm = consts.tile([128, 1], F32, tag="warm")
            nc.scalar.activation(out=warm, in_=zero, func=AF.Exp,
                                 bias=zero[:, 0:1], scale=1.0)
            # dummy-matmul fodder: keeps the PE busy through the initial DMA
            # wait so the HAM clock gate reaches 8/8 before the real work
            # (memset on gpsimd: its icache loads early and the DVE queue
            # stays clear for the first qk relu)
            zwarm = consts.tile([128, 256], BF16, tag="zwarm")
            nc.gpsimd.memset(zwarm, 0.0)

            def warm_burst(n):
                warm_ps = pswk.tile([128, 512], F32, tag="pswk")
                for _ in range(n):
                    nc.tensor.matmul(
                        warm_ps[:, 0:256],
                        zwarm[:, 0:128],
                        zwarm,
                        start=True,
                        stop=True,
                    )

            xb_sb = [None] * bs
            xt_sb = [None] * bs
            x8_sb = [None] * (bs // 2)
            qk_sb = [None] * bs
            qsw = [None] * bs
            v2_sb = [None] * (bs // 2)
            et = [[None] * (CCH // 2) for _ in range(bs)]
            o_sb = [None] * bs
            wvt8 = None

            def load_wvt8():
                # two half-loads so the first v chunks (m<4) unblock early
                nonlocal wvt8
                wvt8 = consts.tile([128, CCH, C], FP8, tag="wvt8")
                nc.sync.dma_start(out=wvt8[:, :, 0:512], in_=wvt_d[:, :, 0:512])
                nc.sync.dma_start(out=wvt8[:, :, 512:C], in_=wvt_d[:, :, 512:C])

            def load_xt(b):
                xt_sb[b] = xtp.tile([128, KCH, C], FP8, tag="xt",
                                    name=f"xt_sb{b}")
                nc.sync.dma_start(out=xt_sb[b], in_=xt_d[b])

            def load_xb(b):
                xb_sb[b] = xbp.tile([128, CCH, N], BF16, tag="xb",
                                    name=f"xb_sb{b}")
                nc.sync.dma_start(out=xb_sb[b], in_=xb_d[b])

            def load_x8(g):
                x8_sb[g] = x8p.tile([128, CCH, 2, N], FP8, tag="x8",
                                    name=f"x8_{g}")
                nc.sync.dma_start(out=x8_sb[g], in_=xb8_d[g])

            def qk_phase(b):
                # q/k projection: psum rows 0:64 = q, 64:128 = k.
                # row-tiled sim needs k at partitions 0:63 and a second copy
                # of q at 64:127 (qsw). Sample 0 gets qsw from a second
                # projection with row-swapped weights (no DMA latency on the
                # head critical path); later samples use sync-queue SBUF
                # copies whose latency hides under the pipeline.
                qk_sb[b] = qkp.tile([128, C], BF16, tag="qk", name=f"qk_sb{b}")
                qsw[b] = qsp.tile([128, C], BF16, tag="qsw", name=f"qsw{b}")
                for cb in range(C // 512):
                    sl = slice(cb * 512, (cb + 1) * 512)
                    qk_ps = pswk.tile([128, 512], F32, tag="pswk")
                    nc.tensor.matmul(
                        qk_ps,
                        wqkt[:, 0],
                        xt_sb[b][:, :, sl],
                        start=True,
                        stop=True,
                        perf_mode=PM.DoubleRow,
                    )
                    # bias+relu on the DVE: keeps the latency-critical relu
                    # out of the ACT FIFO (which is busy draining exp tiles)
                    nc.vector.tensor_scalar(
                        out=qk_sb[b][:, sl], in0=qk_ps,
                        scalar1=tqk[:, 0:1], scalar2=0.0,
                        op0=ALU.add, op1=ALU.max,
                    )
                    if b == 0:
                        qk_ps2 = pswk.tile([128, 512], F32, tag="pswk")
                        nc.tensor.matmul(
                            qk_ps2,
                            wqkt[:, 1],
                            xt_sb[b][:, :, sl],
                            start=True,
                            stop=True,
                            perf_mode=PM.DoubleRow,
                        )
                        nc.vector.tensor_scalar(
                            out=qsw[b][:, sl], in0=qk_ps2,
                            scalar1=tqk[:, 1:2], scalar2=0.0,
                            op0=ALU.add, op1=ALU.max,
                        )
                if b > 0:
                    nc.sync.dma_start(out=qsw[b][0:64, :],
                                      in_=qk_sb[b][64:128, :])
                    nc.sync.dma_start(out=qsw[b][64:128, :],
                                      in_=qk_sb[b][0:64, :])

            def sim_dp(b, dp):
                # transposed sim + exp for one pair of d-chunks:
                # et[b][dp][dd, j, c] = 16*exp(-sim[c, 2dp+j*128+dd]).
                # two d-chunks run concurrently via PE row tiling (K=64 each).
                d0, d1 = 2 * dp, 2 * dp + 1
                psA = psbig.tile([128, C], F32, tag="psbig")
                psB = psbig.tile([128, C], F32, tag="psbig")
                for cb in range(C // 512):
                    sl = slice(cb * 512, (cb + 1) * 512)
                    nc.tensor.matmul(
                        psA[:, sl],
                        qk_sb[b][0:64, d0 * 128:(d0 + 1) * 128],
                        qsw[b][0:64, sl],
                        start=True, stop=True,
                    )
                    nc.tensor.matmul(
                        psB[:, sl],
                        qsw[b][64:128, d1 * 128:(d1 + 1) * 128],
                        qk_sb[b][64:128, sl],
                        start=True, stop=True,
                    )
                e2 = etp.tile([128, 2, C], FP8, tag="et")
                for j, ps in ((0, psA), (1, psB)):
                    # 16*exp(-sim) on the ACT LUT (bias = ln 16)
                    nc.scalar.activation(
                        out=e2[:, j, :], in_=ps, func=AF.Exp,
                        bias=ln16[:, 0:1], scale=-1.0,
                    )
                et[b][dp] = e2

            def v_chunk(g, m):
                # v output chunk m for samples 2g/2g+1 in fp8 DoubleRow:
                # rhs [p, kcpair, j, n] streams 512 virtual columns per
                # weight load. v carries a WV_SCALE factor; the ones-columns
                # put the same factor into Z so U*(1/Z) comes out as
                # alpha*(aff @ v).
                if m == 0:
                    v2_sb[g] = vp.tile([128, CCH, 2, NP], FP8, tag="v",
                                       name=f"v2_{g}")
                    # Z ones-columns (cols N, N+1): tiny strided DVE copy
                    # (the old SWDGE fill flooded the queue with 8B packets)
                    nc.vector.tensor_copy(
                        out=v2_sb[g][:, :, :, N:N + 2], in_=vcol4)
                psv = pswk.tile([128, 2 * N], F32, tag="pswk")
                for kcp in range(CCH // 2):
                    nc.tensor.matmul(
                        psv,
                        wvt8[:, 2 * kcp:2 * kcp + 2, m * 128:(m + 1) * 128],
                        x8_sb[g][:, 2 * kcp:2 * kcp + 2, :, :],
                        start=(kcp == 0),
                        stop=(kcp == CCH // 2 - 1),
                        perf_mode=PM.DoubleRow,
                    )
                nc.vector.tensor_scalar(
                    out=v2_sb[g][:, m, :, 0:N],
                    in0=psv,
                    scalar1=t3[:, m:m + 1],
                    scalar2=0.0,
                    op0=ALU.add,
                    op1=ALU.max,
                )

            def u_chunk(b, m):
                # U = E @ v_ext for output chunk m (col N accumulates
                # (16/alpha)*Z), then out = U * (1/Zcol) + x fused on the DVE
                if m == 0:
                    o_sb[b] = outp.tile([128, CCH, N], BF16, tag="o",
                                        name=f"o_sb{b}")
                u_psw = pswk.tile([128, 512], F32, tag="pswk")
                u_ps = u_psw[:, 0:NP]
                for dp in range(CCH // 2):
                    nc.tensor.matmul(
                        u_ps,
                        et[b][dp][:, :, m * 128:(m + 1) * 128],
                        v2_sb[b // 2][:, 2 * dp:2 * dp + 2, b % 2, :],
                        start=(dp == 0),
                        stop=(dp == CCH // 2 - 1),
                        perf_mode=PM.DoubleRow,
                    )
                rz = smallp.tile([128, 1], F32, tag="rz")
                nc.vector.reciprocal_approx_fast(out=rz, in_=u_ps[:, N:N + 1])
                nc.vector.scalar_tensor_tensor(
                    out=o_sb[b][:, m, :],
                    in0=u_ps[:, 0:N],
                    scalar=rz[:, 0:1],
                    in1=xb_sb[b][:, m, :],
                    op0=ALU.mult,
                    op1=ALU.add,
                )
                if m % 4 == 3:
                    # stream the result out in 4-chunk pieces (each sync
                    # queue trigger costs ~0.65us of queue occupancy)
                    nc.sync.dma_start(
                        out=out_d[b, :, m - 3:m + 1, :],
                        in_=o_sb[b][:, m - 3:m + 1, :],
                    )

            # ---- software-pipelined program order ----
            # sims run at the ACT drain cadence (~2.2us per dp pair, gated
            # by the 2-buf psbig pool); every inter-sim slot carries ~2
            # independent u/v chunks so the PE never idles at the psbig
            # wait, and exp(3,3) -- which gates the serial u3 tail --
            # completes as early as the ACT allows.
            load_xt(0)
            load_x8(0)
            load_wvt8()
            load_xb(0)
            load_xt(1)

            warm_burst(8)
            qk_phase(0)
            warm_burst(3)
            sim_dp(0, 0); v_chunk(0, 0)
            sim_dp(0, 1); v_chunk(0, 1)
            qk_phase(1)
            load_xb(1); load_xt(2)
            sim_dp(0, 2); v_chunk(0, 2); v_chunk(0, 3)
            sim_dp(0, 3); v_chunk(0, 4); v_chunk(0, 5)
            sim_dp(1, 0); v_chunk(0, 6); v_chunk(0, 7)
            sim_dp(1, 1); u_chunk(0, 0); u_chunk(0, 1)
            sim_dp(1, 2); u_chunk(0, 2); u_chunk(0, 3)
            qk_phase(2)
            load_xb(2); load_x8(1); load_xt(3); load_xb(3)
            sim_dp(1, 3); u_chunk(0, 4); u_chunk(0, 5)
            sim_dp(2, 0); u_chunk(0, 6); u_chunk(0, 7)
            sim_dp(2, 1); v_chunk(1, 0); v_chunk(1, 1)
            sim_dp(2, 2); v_chunk(1, 2); v_chunk(1, 3)
            qk_phase(3)
            sim_dp(2, 3); v_chunk(1, 4); v_chunk(1, 5)
            sim_dp(3, 0); v_chunk(1, 6); v_chunk(1, 7)
            sim_dp(3, 1); u_chunk(1, 0); u_chunk(1, 1)
            sim_dp(3, 2); u_chunk(1, 2); u_chunk(1, 3)
            sim_dp(3, 3); u_chunk(1, 4); u_chunk(1, 5)

            for m in range(6, CCH):
                u_chunk(1, m)
            for m in range(CCH):
                u_chunk(2, m)
            for m in range(CCH):
                u_chunk(3, m)

    nc.compile()
    return nc


def _prep_host(x, Wq, Wk, Wv, bn1_g, bn1_b, bn1_m, bn1_v,
               bn2_g, bn2_b, bn2_m, bn2_v, bn3_g, bn3_b, bn3_m, bn3_v):
    f = np.float32
    s1 = (bn1_g / np.sqrt(bn1_v + EPS)).astype(f)
    t1 = (bn1_b - s1 * bn1_m).astype(f)
    s2 = (bn2_g / np.sqrt(bn2_v + EPS)).astype(f)
    t2 = (bn2_b - s2 * bn2_m).astype(f)
    s3 = (bn3_g / np.sqrt(bn3_v + EPS)).astype(f)
    t3 = ((bn3_b - s3 * bn3_m) * WV_SCALE).astype(f)

    wq1 = (Wq * s1[:, None]).astype(f)
    wk2 = (Wk * s2[:, None]).astype(f)
    # lhsT layout [p(=n local), v, kc, o], fp8: v=0 is [q; k] rows, v=1 the
    # row-swapped [k; q] used to produce sample 0's qsw directly
    wqkt = np.ascontiguousarray(np.stack([
        np.concatenate([wq1, wk2], axis=0).T.reshape(KCH, 128, 128),
        np.concatenate([wk2, wq1], axis=0).T.reshape(KCH, 128, 128),
    ], axis=1).transpose(2, 1, 0, 3)).astype(F8NP)
    tqk = np.stack([np.concatenate([t1, t2]),
                    np.concatenate([t2, t1])], axis=1).astype(f)

    wv2 = (Wv * (s3 * WV_SCALE)[:, None]).astype(f)
    # lhsT layout [p(=ci local), kc, co], fp8
    wvt8 = np.ascontiguousarray(
        wv2.T.reshape(CCH, 128, C).transpose(1, 0, 2)).astype(F8NP)
    t3r = np.ascontiguousarray(t3.reshape(CCH, 128).T)

    x = np.asarray(x, dtype=f)
    # [b, p(=c local), kc, n]
    xq = x.reshape(B, CCH, 128, N).transpose(0, 2, 1, 3)
    xb = np.ascontiguousarray(xq).astype(BFNP)
    # [g, p, kc, j, n] fp8 pairs for the DoubleRow v matmul
    xb8 = np.ascontiguousarray(
        xq.reshape(B // 2, 2, 128, CCH, N).transpose(0, 2, 3, 1, 4)
    ).astype(F8NP)
    # [b, p(=n local), kc, c], fp8 for the DoubleRow qk matmul
    xt = np.ascontiguousarray(
        x.transpose(0, 2, 1).reshape(B, KCH, 128, C).transpose(0, 2, 1, 3)
    ).astype(F8NP)
    return xb, xb8, xt, wqkt, tqk, wvt8, t3r


def kernel(x, Wq, Wk, Wv,
           bn1_g, bn1_b, bn1_m, bn1_v,
           bn2_g, bn2_b, bn2_m, bn2_v,
           bn3_g, bn3_b, bn3_m, bn3_v,
           alpha):
    global LAST_RESULTS
    args = [np.asarray(a, dtype=np.float32) for a in (
        x, Wq, Wk, Wv, bn1_g, bn1_b, bn1_m, bn1_v,
        bn2_g, bn2_b, bn2_m, bn2_v, bn3_g, bn3_b, bn3_m, bn3_v)]
    alpha_val = float(np.asarray(alpha).reshape(-1)[0])
    if alpha_val == 0.0:
        return np.asarray(x, dtype=np.float32).copy()

    xb, xb8, xt, wqkt, tqk, wvt8, t3r = _prep_host(*args)
    # the Z column accumulates (WV_SCALE/alpha)*Z, so U*(1/Zcol) yields
    # alpha*(aff @ v) with both the fp8 weight scale and alpha folded in
    vcol = np.full((128, CCH, 2, 2), WV_SCALE / alpha_val, dtype=BFNP)

    if "nc" not in _NC_CACHE:
        _NC_CACHE["nc"] = _build()
    nc = _NC_CACHE["nc"]

    in_maps = []
    for cid in range(NCORES):
        sl = slice(cid * BS, (cid + 1) * BS)
        slg = slice(cid * (BS // 2), (cid + 1) * (BS // 2))
        in_maps.append({
            "xb_in": np.ascontiguousarray(xb[sl]),
            "xb8_in": np.ascontiguousarray(xb8[slg]),
            "xt_in": np.ascontiguousarray(xt[sl]),
            "wqkt": wqkt,
            "tqk": tqk,
            "wvt": wvt8,
            "t3": t3r,
            "vcol": vcol,
        })

    trace = bool(int(os.environ.get("KERNEL_TRACE", "0")))
    tmpdir = os.environ.get("KERNEL_TRACE_DIR") or None
    res = run_bass_kernel_spmd(
        nc, in_maps, core_ids=list(range(NCORES)), trace=trace, tmpdir=tmpdir
    )
    LAST_RESULTS = res

    y = np.concatenate(
        [np.asarray(res.results[cid]["y_out"], dtype=np.float32)
         for cid in range(NCORES)], axis=0)
    y = y.transpose(0, 2, 1, 3).reshape(B, C, N)
    return np.ascontiguousarray(y)


# revision 20
# speedup vs baseline: 1.3107x; 1.0041x over previous
"""Trainium2 Bass kernel for the CAA (channel-affinity attention) module.

Reference computation per sample b (C=1024 channels, N=256 positions):
    x_hat = x^T                              (N, C)
    q = relu(BN1(Wq @ x_hat))                (64, C)
    k = relu(BN2(Wk @ x_hat))                (64, C)
    sim[c, d] = sum_o k[o, c] * q[o, d]      (C, C)
    aff = softmax(rowmax(sim) - sim, axis d) == softmax(-sim, axis d)
    v = relu(BN3(Wv @ x))                    (C, N)
    out = alpha * (aff @ v) + x              (C, N)

Device-side strategy (pure data parallel, 4 samples per core x 8 cores):
  * BN folded into weights/bias on the host.
  * sim is computed TRANSPOSED (d on partitions, c on free) so the exp(-sim)
    tiles feed the aff @ v contraction directly as matmul lhsT.
  * sim uses 2-way PE row tiling: the K=64 contraction only fills half the
    array, so two d-chunks run concurrently in rows 0-63 / 64-127 (needs k
    copied to partitions 0-63 and q to 64-127 -- one SBUF-SBUF DMA each).
  * the qk projection, v = relu(Wv'x + t3), and U = E @ v_ext all run in
    fp8 (e4m3) with DoubleRow perf mode, which packs two contraction rows
    per PE cell (2x throughput). v packs two samples side by side in the
    moving operand so each matmul streams 512 virtual columns per weight
    load; U pairs adjacent d-chunks via [p, 2, free] access patterns.
  * exp tiles are 16*exp(-sim) in fp8 (ACT LUT, bias=ln16): the 16x keeps
    e4m3 from underflowing whole softmax rows (row-min sim <= ~10), and
    the same factor enters Z through the 16/alpha ones-columns, so it
    cancels in U * (1/Zcol). Wv carries a further 16x for the same reason.
  * epilogues: v bias+relu and the fused U*(1/Z)+x reside on the DVE
    (reciprocal_approx_fast for 1/Z); exp owns the ACT; the PE clock is
    kept warm through the initial DMA wait by dummy matmuls.
  * the residual add reads the bf16 copy of x; output is stored bf16.

Scheduling (v2): all weights load FIRST on the HW sync queue (the old
SWDGE path delivered wqkt at ~17us and stalled the first projection), and
the per-sample phases are software-pipelined at chunk granularity: sim
dp-chunks are interleaved with u/v m-chunks in program order so the PE
engine queue always has >~2.3us of independent work between consecutive
sim chunks (psbig has 2 PSUM bufs; sim chunk t+1 must wait for ACT to
drain chunk t's exps -- FIFO queues stall everything behind it otherwise).
"""

import os
import sys

import numpy as np

_REPO = "/opt/trn_rl_repo"
if _REPO not in sys.path:
    sys.path.insert(0, _REPO)

import ml_dtypes  # noqa: E402

import concourse.bacc as bacc  # noqa: E402
import concourse.tile as tile  # noqa: E402
from concourse import mybir  # noqa: E402
from concourse.bass_utils import run_bass_kernel_spmd  # noqa: E402

F32 = mybir.dt.float32
BF16 = mybir.dt.bfloat16
FP8 = mybir.dt.float8e4
AF = mybir.ActivationFunctionType
ALU = mybir.AluOpType
PM = mybir.MatmulPerfMode
BFNP = ml_dtypes.bfloat16
F8NP = ml_dtypes.float8_e4m3

B, C, N = 32, 1024, 256
DQ = 64
NCORES = 8
BS = B // NCORES  # samples per core
CCH = C // 128    # chunks of the channel dim
KCH = N // 128    # chunks of the position dim (qk contraction)
EPS = 1e-5
WV_SCALE = 16.0   # keeps fp8 Wv weights in e4m3 normal range
E_SCALE_LN = 2.772588722239781  # ln(16): exp tiles carry a 16x factor
NP = 264          # padded v free size (fp8 DoubleRow AP needs %16 strides)
LAST_RESULTS = None  # BassKernelResults of the most recent run
_NC_CACHE = {}


def _build(bs: int = BS):
    nc = bacc.Bacc("TRN2", target_bir_lowering=False, debug=False)

    xb_d = nc.dram_tensor("xb_in", (bs, 128, CCH, N), BF16, kind="ExternalInput")
    xt_d = nc.dram_tensor("xt_in", (bs, 128, KCH, C), FP8, kind="ExternalInput")
    # fp8 x with two samples interleaved at the n level: [g, p, kc, j, n]
    xb8_d = nc.dram_tensor("xb8_in", (bs // 2, 128, CCH, 2, N), FP8,
                           kind="ExternalInput")
    # [p, v, kc, o]: v=0 normal ([q; k] rows), v=1 swapped ([k; q] rows) --
    # the swapped projection writes sample 0's qsw layout directly
    wqkt_d = nc.dram_tensor("wqkt", (128, 2, KCH, 128), FP8, kind="ExternalInput")
    tqk_d = nc.dram_tensor("tqk", (128, 2), F32, kind="ExternalInput")
    wvt_d = nc.dram_tensor("wvt", (128, CCH, C), FP8, kind="ExternalInput")
    t3_d = nc.dram_tensor("t3", (128, CCH), F32, kind="ExternalInput")
    vcol_d = nc.dram_tensor("vcol", (128, CCH, 2, 2), BF16, kind="ExternalInput")
    out_d = nc.dram_tensor("y_out", (bs, 128, CCH, N), BF16, kind="ExternalOutput")

    with tile.TileContext(nc) as tc:
        with (
            tc.tile_pool(name="consts", bufs=1) as consts,
            tc.tile_pool(name="xbp", bufs=4) as xbp,
            tc.tile_pool(name="xtp", bufs=2) as xtp,
            tc.tile_pool(name="x8p", bufs=2) as x8p,
            tc.tile_pool(name="qkp", bufs=2) as qkp,
            tc.tile_pool(name="qsp", bufs=2) as qsp,
            tc.tile_pool(name="etp", bufs=10) as etp,
            tc.tile_pool(name="vp", bufs=2) as vp,
            tc.tile_pool(name="outp", bufs=2) as outp,
            tc.tile_pool(name="smallp", bufs=8) as smallp,
            # psbig holds 3 sim psum pairs so the sim matmuls never gate on
            # the ACT exp drain (with 2 bufs the exp->matmul->exp round trip
            # added ~1us per dp pair); qk/v/u share one single-bank pool
            tc.tile_pool(name="psbig", bufs=3, space="PSUM") as psbig,
            tc.tile_pool(name="pswk", bufs=2, space="PSUM") as pswk,
        ):
            # each DMA trigger occupies its queue ~0.65us, so the head loads
            # are spread across queues: sync carries wqkt + the big x
            # tensors (in need-order), the scalar queue the tiny biases
            wqkt = consts.tile([128, 2, KCH, 128], FP8, tag="wqkt")
            nc.sync.dma_start(out=wqkt, in_=wqkt_d[:])
            tqk = consts.tile([128, 2], F32, tag="tqk")
            nc.scalar.dma_start(out=tqk, in_=tqk_d[:])
            t3 = consts.tile([128, CCH], F32, tag="t3")
            nc.scalar.dma_start(out=t3, in_=t3_d[:])
            vcol4 = consts.tile([128, CCH, 2, 2], BF16, tag="vcol")
            nc.scalar.dma_start(out=vcol4, in_=vcol_d[:])

            zero = consts.tile([128, 1], F32, tag="zero")
            nc.gpsimd.memset(zero, 0.0)
            ln16 = consts.tile([128, 1], F32, tag="ln16")
            nc.gpsimd.memset(ln16, E_SCALE_LN)
            # touch the activation table early so the lazy ACT_TABLE_LOAD
            # doesn't delay the first (critical-path) exp
            warm = consts.tile([128, 1], F32, tag="warm")
            nc.scalar.activation(out=warm, in_=zero, func=AF.Exp,
                                 bias=zero[:, 0:1], scale=1.0)
            # dummy-matmul fodder: keeps the PE busy through the initial DMA
            # wait so the HAM clock gate reaches 8/8 before the real work
            # (memset on gpsimd: its icache loads early and the DVE queue
            # stays clear for the first qk relu)
            zwarm = consts.tile([128, 256], BF16, tag="zwarm")
            nc.gpsimd.memset(zwarm, 0.0)

            def warm_burst(n):
                warm_ps = pswk.tile([128, 512], F32, tag="pswk")
                for _ in range(n):
                    nc.tensor.matmul(
                        warm_ps[:, 0:256],
                        zwarm[:, 0:128],
                        zwarm,
                        start=True,
                        stop=True,
                    )

            xb_sb = [None] * bs
            xt_sb = [None] * bs
            x8_sb = [None] * (bs // 2)
            qk_sb = [None] * bs
            qsw = [None] * bs
            v2_sb = [None] * (bs // 2)
            et = [[None] * (CCH // 2) for _ in range(bs)]
            o_sb = [None] * bs
            wvt8 = None

            def load_wvt8():
                # two half-loads so the first v chunks (m<4) unblock early
                nonlocal wvt8
                wvt8 = consts.tile([128, CCH, C], FP8, tag="wvt8")
                nc.sync.dma_start(out=wvt8[:, :, 0:512], in_=wvt_d[:, :, 0:512])
                nc.sync.dma_start(out=wvt8[:, :, 512:C], in_=wvt_d[:, :, 512:C])

            def load_xt(b):
                xt_sb[b] = xtp.tile([128, KCH, C], FP8, tag="xt",
                                    name=f"xt_sb{b}")
                nc.sync.dma_start(out=xt_sb[b], in_=xt_d[b])

            def load_xb(b):
                xb_sb[b] = xbp.tile([128, CCH, N], BF16, tag="xb",
                                    name=f"xb_sb{b}")
                nc.sync.dma_start(out=xb_sb[b], in_=xb_d[b])

            def load_x8(g):
                x8_sb[g] = x8p.tile([128, CCH, 2, N], FP8, tag="x8",
                                    name=f"x8_{g}")
                nc.sync.dma_start(out=x8_sb[g], in_=xb8_d[g])

            def qk_phase(b):
                # q/k projection: psum rows 0:64 = q, 64:128 = k.
                # row-tiled sim needs k at partitions 0:63 and a second copy
                # of q at 64:127 (qsw). Sample 0 gets qsw from a second
                # projection with row-swapped weights (no DMA latency on the
                # head critical path); later samples use sync-queue SBUF
                # copies whose latency hides under the pipeline.
                qk_sb[b] = qkp.tile([128, C], BF16, tag="qk", name=f"qk_sb{b}")
                qsw[b] = qsp.tile([128, C], BF16, tag="qsw", name=f"qsw{b}")
                for cb in range(C // 512):
                    sl = slice(cb * 512, (cb + 1) * 512)
                    qk_ps = pswk.tile([128, 512], F32, tag="pswk")
                    nc.tensor.matmul(
                        qk_ps,
                        wqkt[:, 0],
                        xt_sb[b][:, :, sl],
                        start=True,
                        stop=True,
                        perf_mode=PM.DoubleRow,
                    )
                    # bias+relu on the DVE: keeps the latency-critical relu
                    # out of the ACT FIFO (which is busy draining exp tiles)
                    nc.vector.tensor_scalar(
                        out=qk_sb[b][:, sl], in0=qk_ps,
                        scalar1=tqk[:, 0:1], scalar2=0.0,
                        op0=ALU.add, op1=ALU.max,
                    )
                    if b == 0:
                        qk_ps2 = pswk.tile([128, 512], F32, tag="pswk")
                        nc.tensor.matmul(
                            qk_ps2,
                            wqkt[:, 1],
                            xt_sb[b][:, :, sl],
                            start=True,
                            stop=True,
                            perf_mode=PM.DoubleRow,
                        )
                        nc.vector.tensor_scalar(
                            out=qsw[b][:, sl], in0=qk_ps2,
                            scalar1=tqk[:, 1:2], scalar2=0.0,
                            op0=ALU.add, op1=ALU.max,
                        )
                if b > 0:
                    nc.sync.dma_start(out=qsw[b][0:64, :],
                                      in_=qk_sb[b][64:128, :])
                    nc.sync.dma_start(out=qsw[b][64:128, :],
                                      in_=qk_sb[b][0:64, :])

            def sim_dp(b, dp):
                # transposed sim + exp for one pair of d-chunks:
                # et[b][dp][dd, j, c] = 16*exp(-sim[c, 2dp+j*128+dd]).
                # two d-chunks run concurrently via PE row tiling (K=64 each).
                d0, d1 = 2 * dp, 2 * dp + 1
                psA = psbig.tile([128, C], F32, tag="psbig")
                psB = psbig.tile([128, C], F32, tag="psbig")
                for cb in range(C // 512):
                    sl = slice(cb * 512, (cb + 1) * 512)
                    nc.tensor.matmul(
                        psA[:, sl],
                        qk_sb[b][0:64, d0 * 128:(d0 + 1) * 128],
                        qsw[b][0:64, sl],
                        start=True, stop=True,
                    )
                    nc.tensor.matmul(
                        psB[:, sl],
                        qsw[b][64:128, d1 * 128:(d1 + 1) * 128],
                        qk_sb[b][64:128, sl],
                        start=True, stop=True,
                    )
                # et layout is the DoubleRowSwInterleave weight format: per
                # m-group of 256 bytes, [A127 B127 A126 B126 ... A0 B0]
                # (A = d-chunk 2dp from psA, B = 2dp+1 from psB, columns
                # reversed). The U-phase LDWEIGHTS then reads contiguously.
                e2 = etp.tile([128, CCH, 256], FP8, tag="et")
                for off, ps in ((254, psA), (255, psB)):
                    # 16*exp(-sim) on the ACT LUT (bias = ln 16)
                    nc.scalar.activation(
                        out=e2[:, :, off::-2], in_=ps, func=AF.Exp,
                        bias=ln16[:, 0:1], scale=-1.0,
                    )
                et[b][dp] = e2

            def v_chunk(g, m):
                # v output chunk m for samples 2g/2g+1 in fp8 DoubleRow:
                # rhs [p, kcpair, j, n] streams 512 virtual columns per
                # weight load. v carries a WV_SCALE factor; the ones-columns
                # put the same factor into Z so U*(1/Z) comes out as
                # alpha*(aff @ v).
                if m == 0:
                    v2_sb[g] = vp.tile([128, CCH, 2, NP], FP8, tag="v",
                                       name=f"v2_{g}")
                    # Z ones-columns (cols N, N+1): tiny strided DVE copy
                    # (the old SWDGE fill flooded the queue with 8B packets)
                    nc.vector.tensor_copy(
                        out=v2_sb[g][:, :, :, N:N + 2], in_=vcol4)
                psv = pswk.tile([128, 2 * N], F32, tag="pswk")
                for kcp in range(CCH // 2):
                    nc.tensor.matmul(
                        psv,
                        wvt8[:, 2 * kcp:2 * kcp + 2, m * 128:(m + 1) * 128],
                        x8_sb[g][:, 2 * kcp:2 * kcp + 2, :, :],
                        start=(kcp == 0),
                        stop=(kcp == CCH // 2 - 1),
                        perf_mode=PM.DoubleRow,
                    )
                nc.vector.tensor_scalar(
                    out=v2_sb[g][:, m, :, 0:N],
                    in0=psv,
                    scalar1=t3[:, m:m + 1],
                    scalar2=0.0,
                    op0=ALU.add,
                    op1=ALU.max,
                )

            def u_chunk(b, m):
                # U = E @ v_ext for output chunk m (col N accumulates
                # (16/alpha)*Z), then out = U * (1/Zcol) + x fused on the DVE
                if m == 0:
                    o_sb[b] = outp.tile([128, CCH, N], BF16, tag="o",
                                        name=f"o_sb{b}")
                u_psw = pswk.tile([128, 512], F32, tag="pswk")
                u_ps = u_psw[:, 0:NP]
                for dp in range(CCH // 2):
                    nc.tensor.matmul(
                        u_ps,
                        et[b][dp][:, m, :],
                        v2_sb[b // 2][:, 2 * dp:2 * dp + 2, b % 2, :],
                        start=(dp == 0),
                        stop=(dp == CCH // 2 - 1),
                        perf_mode=PM.DoubleRowSwInterleave,
                    )
                rz = smallp.tile([128, 1], F32, tag="rz")
                nc.vector.reciprocal_approx_fast(out=rz, in_=u_ps[:, N:N + 1])
                nc.vector.scalar_tensor_tensor(
                    out=o_sb[b][:, m, :],
                    in0=u_ps[:, 0:N],
                    scalar=rz[:, 0:1],
                    in1=xb_sb[b][:, m, :],
                    op0=ALU.mult,
                    op1=ALU.add,
                )
                if m % 4 == 3:
                    # stream the result out in 4-chunk pieces (each sync
                    # queue trigger costs ~0.65us of queue occupancy)
                    nc.sync.dma_start(
                        out=out_d[b, :, m - 3:m + 1, :],
                        in_=o_sb[b][:, m - 3:m + 1, :],
                    )

            # ---- software-pipelined program order ----
            # sims run at the ACT drain cadence (~2.2us per dp pair, gated
            # by the 2-buf psbig pool); every inter-sim slot carries ~2
            # independent u/v chunks so the PE never idles at the psbig
            # wait, and exp(3,3) -- which gates the serial u3 tail --
            # completes as early as the ACT allows.
            load_xt(0)
            load_x8(0)
            load_wvt8()
            load_xb(0)
            load_xt(1)

            warm_burst(8)
            qk_phase(0)
            warm_burst(3)
            sim_dp(0, 0); v_chunk(0, 0)
            sim_dp(0, 1); v_chunk(0, 1)
            qk_phase(1)
            load_xb(1); load_xt(2)
            sim_dp(0, 2); v_chunk(0, 2); v_chunk(0, 3)
            sim_dp(0, 3); v_chunk(0, 4); v_chunk(0, 5)
            sim_dp(1, 0); v_chunk(0, 6); v_chunk(0, 7)
            sim_dp(1, 1); u_chunk(0, 0); u_chunk(0, 1)
            sim_dp(1, 2); u_chunk(0, 2); u_chunk(0, 3)
            qk_phase(2)
            load_xb(2); load_x8(1); load_xt(3); load_xb(3)
            sim_dp(1, 3); u_chunk(0, 4); u_chunk(0, 5)
            sim_dp(2, 0); u_chunk(0, 6); u_chunk(0, 7)
            sim_dp(2, 1); v_chunk(1, 0); v_chunk(1, 1)
            sim_dp(2, 2); v_chunk(1, 2); v_chunk(1, 3)
            qk_phase(3)
            sim_dp(2, 3); v_chunk(1, 4); v_chunk(1, 5)
            sim_dp(3, 0); v_chunk(1, 6); v_chunk(1, 7)
            sim_dp(3, 1); u_chunk(1, 0); u_chunk(1, 1)
            sim_dp(3, 2); u_chunk(1, 2); u_chunk(1, 3)
            sim_dp(3, 3); u_chunk(1, 4); u_chunk(1, 5)

            for m in range(6, CCH):
                u_chunk(1, m)
            for m in range(CCH):
                u_chunk(2, m)
            for m in range(CCH):
                u_chunk(3, m)

    nc.compile()
    return nc


def _prep_host(x, Wq, Wk, Wv, bn1_g, bn1_b, bn1_m, bn1_v,
               bn2_g, bn2_b, bn2_m, bn2_v, bn3_g, bn3_b, bn3_m, bn3_v):
    f = np.float32
    s1 = (bn1_g / np.sqrt(bn1_v + EPS)).astype(f)
    t1 = (bn1_b - s1 * bn1_m).astype(f)
    s2 = (bn2_g / np.sqrt(bn2_v + EPS)).astype(f)
    t2 = (bn2_b - s2 * bn2_m).astype(f)
    s3 = (bn3_g / np.sqrt(bn3_v + EPS)).astype(f)
    t3 = ((bn3_b - s3 * bn3_m) * WV_SCALE).astype(f)

    wq1 = (Wq * s1[:, None]).astype(f)
    wk2 = (Wk * s2[:, None]).astype(f)
    # lhsT layout [p(=n local), v, kc, o], fp8: v=0 is [q; k] rows, v=1 the
    # row-swapped [k; q] used to produce sample 0's qsw directly
    wqkt = np.ascontiguousarray(np.stack([
        np.concatenate([wq1, wk2], axis=0).T.reshape(KCH, 128, 128),
        np.concatenate([wk2, wq1], axis=0).T.reshape(KCH, 128, 128),
    ], axis=1).transpose(2, 1, 0, 3)).astype(F8NP)
    tqk = np.stack([np.concatenate([t1, t2]),
                    np.concatenate([t2, t1])], axis=1).astype(f)

    wv2 = (Wv * (s3 * WV_SCALE)[:, None]).astype(f)
    # lhsT layout [p(=ci local), kc, co], fp8
    wvt8 = np.ascontiguousarray(
        wv2.T.reshape(CCH, 128, C).transpose(1, 0, 2)).astype(F8NP)
    t3r = np.ascontiguousarray(t3.reshape(CCH, 128).T)

    x = np.asarray(x, dtype=f)
    # [b, p(=c local), kc, n]
    xq = x.reshape(B, CCH, 128, N).transpose(0, 2, 1, 3)
    xb = np.ascontiguousarray(xq).astype(BFNP)
    # [g, p, kc, j, n] fp8 pairs for the DoubleRow v matmul
    xb8 = np.ascontiguousarray(
        xq.reshape(B // 2, 2, 128, CCH, N).transpose(0, 2, 3, 1, 4)
    ).astype(F8NP)
    # [b, p(=n local), kc, c], fp8 for the DoubleRow qk matmul
    xt = np.ascontiguousarray(
        x.transpose(0, 2, 1).reshape(B, KCH, 128, C).transpose(0, 2, 1, 3)
    ).astype(F8NP)
    return xb, xb8, xt, wqkt, tqk, wvt8, t3r


def kernel(x, Wq, Wk, Wv,
           bn1_g, bn1_b, bn1_m, bn1_v,
           bn2_g, bn2_b, bn2_m, bn2_v,
           bn3_g, bn3_b, bn3_m, bn3_v,
           alpha):
    global LAST_RESULTS
    args = [np.asarray(a, dtype=np.float32) for a in (
        x, Wq, Wk, Wv, bn1_g, bn1_b, bn1_m, bn1_v,
        bn2_g, bn2_b, bn2_m, bn2_v, bn3_g, bn3_b, bn3_m, bn3_v)]
    alpha_val = float(np.asarray(alpha).reshape(-1)[0])
    if alpha_val == 0.0:
        return np.asarray(x, dtype=np.float32).copy()

    xb, xb8, xt, wqkt, tqk, wvt8, t3r = _prep_host(*args)
    # the Z column accumulates (WV_SCALE/alpha)*Z, so U*(1/Zcol) yields
    # alpha*(aff @ v) with both the fp8 weight scale and alpha folded in
    vcol = np.full((128, CCH, 2, 2), WV_SCALE / alpha_val, dtype=BFNP)

    if "nc" not in _NC_CACHE:
        _NC_CACHE["nc"] = _build()
    nc = _NC_CACHE["nc"]

    in_maps = []
    for cid in range(NCORES):
        sl = slice(cid * BS, (cid + 1) * BS)
        slg = slice(cid * (BS // 2), (cid + 1) * (BS // 2))
        in_maps.append({
            "xb_in": np.ascontiguousarray(xb[sl]),
            "xb8_in": np.ascontiguousarray(xb8[slg]),
            "xt_in": np.ascontiguousarray(xt[sl]),
            "wqkt": wqkt,
            "tqk": tqk,
            "wvt": wvt8,
            "t3": t3r,
            "vcol": vcol,
        })

    trace = bool(int(os.environ.get("KERNEL_TRACE", "0")))
    tmpdir = os.environ.get("KERNEL_TRACE_DIR") or None
    res = run_bass_kernel_spmd(
        nc, in_maps, core_ids=list(range(NCORES)), trace=trace, tmpdir=tmpdir
    )
    LAST_RESULTS = res

    y = np.concatenate(
        [np.asarray(res.results[cid]["y_out"], dtype=np.float32)
         for cid in range(NCORES)], axis=0)
    y = y.transpose(0, 2, 1, 3).reshape(B, C, N)
    return np.ascontiguousarray(y)
